# revision 1
# baseline (speedup 1.0000x reference)
"""GAT model Bass/Tile kernel for TRN2 (self-contained).

Per-core layout (core m of 8): 512 graphs, processed as 256 "pairs"
(2 graphs = 128 nodes, 112 edges per pair). All three GAT layers +
pooling + MLP run per pair entirely on-chip; only x^T, edge data and
small packed weights are read from HBM, and [2, 16] outputs per pair
are written back.

Orientation scheme per layer:
  stationary = prev activations feature-major [c_in, 128v]
  MM1  -> h/As/Ad/As+Ad node-major psum [128v, 280]
  gathers via one-hot matmuls (S v-major stationary; D^T edge-major)
  scatter -> feature-major psum [c, 128v] -> relu(+bias) -> next input
"""
import numpy as np
from contextlib import ExitStack

import concourse.bass as bass
import concourse.tile as tile
from concourse import bacc, mybir
from concourse.bass_utils import run_bass_kernel_spmd

F32 = mybir.dt.float32
I32 = mybir.dt.int32

B, A, OBS = 4096, 8, 56
P = 64                      # nodes per graph
H, HID, HC = 8, 32, 256
IN, OUT = 16, 2
NCORES = 8
GPC = B // NCORES           # graphs per core
EPP = 2 * OBS               # edges per pair = 112
ALU = mybir.AluOpType
ACTF = mybir.ActivationFunctionType


def build(npairs: int, vdt=mybir.dt.bfloat16, num_devices: int = NCORES):
    """Build the per-core SPMD program for `npairs` graph-pairs."""
    nc = bacc.Bacc("TRN2", target_bir_lowering=False, debug=False,
                   num_devices=num_devices)

    NP = npairs
    dram = {}

    def din(name, shape, dt):
        dram[name] = nc.dram_tensor(name, shape, dt, kind="ExternalInput").ap()
        return dram[name]

    xt = din("xt", [IN, NP * 128], vdt)
    esrc = din("esrc", [EPP, NP], F32)
    edst = din("edst", [EPP, NP], F32)
    eattr = din("eattr", [EPP, NP], F32)
    eattr_v = din("eattr_v", [EPP, NP], vdt)
    waug1 = din("waug1", [IN, 280], vdt)
    waug2 = din("waug2", [128, 560], vdt)   # [256,280] packed 2 chunks wide
    waug3 = din("waug3", [128, 560], vdt)
    web112 = din("web112", [EPP, 3 * H], F32)
    web128 = din("web128", [128, 3 * H], F32)
    bvec = din("bvec", [128, 6], F32)
    fc1a = din("fc1a", [128, HC], vdt)      # [256,128] packed 2 chunks wide
    fc1g = din("fc1g", [128, HC], vdt)
    fc1b = din("fc1b", [128, 1], F32)
    fc2w = din("fc2w", [128, OUT], vdt)
    fc2b = din("fc2b", [OUT, 1], F32)
    ident = din("ident", [128, 128], vdt)
    iota = din("iota", [EPP, 128], F32)
    ones1 = din("ones1", [EPP, 1], vdt)

    out_d = nc.dram_tensor("out", [OUT, NP * 16], F32, kind="ExternalOutput").ap()

    with tile.TileContext(nc) as tc, ExitStack() as ctx:
        cpool = ctx.enter_context(tc.tile_pool(name="const", bufs=1))
        wk = ctx.enter_context(tc.tile_pool(name="work", bufs=2))
        ps = ctx.enter_context(tc.tile_pool(name="psum", bufs=1, space="PSUM"))

        # ---- load constants into SBUF ----
        def cload(ap, tag):
            t = cpool.tile(list(ap.shape), ap.dtype, tag=tag)
            nc.sync.dma_start(t[:], ap[:, :])
            return t

        c_w1 = cload(waug1, "w1")
        c_w2 = cload(waug2, "w2")          # [256, 280] -> 2 chunks via slicing
        c_w3 = cload(waug3, "w3")
        c_web112 = cload(web112, "web112")
        c_web128 = cload(web128, "web128")
        c_bvec = cload(bvec, "bvec")
        c_fc1a = cload(fc1a, "fc1a")
        c_fc1g = cload(fc1g, "fc1g")
        c_fc1b = cload(fc1b, "fc1b")
        c_fc2w = cload(fc2w, "fc2w")
        c_fc2b = cload(fc2b, "fc2b")
        c_id = cload(ident, "ident")
        c_iota = cload(iota, "iota")
        c_ones1 = cload(ones1, "ones1")
        c_esrc = cload(esrc, "esrc")
        c_edst = cload(edst, "edst")
        c_ea = cload(eattr, "eattr")
        c_eav = cload(eattr_v, "eattr_v")

        out_acc = cpool.tile([OUT, NP * 16], F32, tag="out_acc")

        # waug chunks per layer: list of (sbuf AP [k<=128, 280])
        wchunks = {
            1: [c_w1[:, :]],
            2: [c_w2[:, 0:280], c_w2[:, 280:560]],
            3: [c_w3[:, 0:280], c_w3[:, 280:560]],
        }

        zbat = None
        for p in range(NP):
            po = p % 8
            if po == 0:
                zbat = wk.tile([128, 128], vdt, tag="zbat")
                if NP - p < 8:
                    nc.vector.memset(zbat[:], 0.0)

            # ---- per-pair edge structure ----
            st = wk.tile([EPP, 128], vdt, tag="st")
            nc.vector.tensor_scalar(st[:], c_iota[:], c_esrc[:, p:p + 1], None,
                                    ALU.is_equal)
            dtt = wk.tile([EPP, 128], vdt, tag="dtt")
            nc.vector.tensor_scalar(dtt[:], c_iota[:], c_edst[:, p:p + 1], None,
                                    ALU.is_equal)
            tr_ps = ps.tile([128, 224], vdt, tag="transp")
            nc.tensor.transpose(tr_ps[:, 0:112], st[:], c_id[0:EPP, 0:EPP])
            nc.tensor.transpose(tr_ps[:, 112:224], dtt[:], c_id[0:EPP, 0:EPP])
            sblk = wk.tile([128, EPP], vdt, tag="sblk")
            nc.vector.tensor_copy(sblk[:], tr_ps[:, 0:112])
            dblk = wk.tile([128, EPP], vdt, tag="dblk")
            nc.vector.tensor_copy(dblk[:], tr_ps[:, 112:224])

            # xt for layer 1 (feature-major stationary)
            x0 = wk.tile([IN, 128], vdt, tag="x0")
            nc.sync.dma_start(x0[:], xt[:, p * 128:(p + 1) * 128])

            ea_loop = None     # [128,1] f32, built in layer 1
            prevT = [x0[:, :]]  # list of [k,128] stationary chunks
            hT_out = None

            for li in (1, 2, 3):
                wch = wchunks[li]
                self_loops = li > 1

                # ---- MM1: node-major h | As | Ad | As+Ad ----
                h_ps = ps.tile([128, 280], F32, tag="hps")
                for kc, (sta, wc) in enumerate(zip(prevT, wch)):
                    nc.tensor.matmul(h_ps[:], sta, wc,
                                     start=(kc == 0), stop=(kc == len(wch) - 1))

                hAs = wk.tile([128, 272], vdt, tag="hAs")
                nc.vector.tensor_copy(hAs[:], h_ps[:, 0:272])

                if li == 1:
                    # cnt/easum scatter (edge-structure only; reused later)
                    cnt_ps = ps.tile([128, 2], F32, tag="dencnt")
                    nc.tensor.matmul(cnt_ps[:, 0:1], dtt[:], c_ones1[:, :],
                                     start=True, stop=True)
                    nc.tensor.matmul(cnt_ps[:, 1:2], dtt[:],
                                     c_eav[:, p:p + 1], start=True, stop=True)
                    cntm = wk.tile([128, 1], F32, tag="cntm")
                    nc.vector.tensor_scalar(cntm[:], cnt_ps[:, 0:1], 1.0, None,
                                            ALU.max)
                    rc = wk.tile([128, 1], F32, tag="rc")
                    nc.vector.reciprocal(rc[:], cntm[:])
                    ea_loop = wk.tile([128, 1], F32, tag="ea_loop")
                    nc.vector.tensor_tensor(ea_loop[:], cnt_ps[:, 1:2], rc[:],
                                            ALU.mult)

                p_self = None
                if self_loops:
                    selfae = wk.tile([128, H], F32, tag="selfae")
                    nc.vector.tensor_scalar(
                        selfae[:], c_web128[:, (li - 1) * H:li * H],
                        ea_loop[:], None, ALU.mult)
                    ls = wk.tile([128, H], F32, tag="ls")
                    nc.vector.scalar_tensor_tensor(
                        ls[:], h_ps[:, 272:280], 1.0, selfae[:],
                        ALU.mult, ALU.add)
                    ls2 = wk.tile([128, H], F32, tag="ls2")
                    nc.vector.scalar_tensor_tensor(
                        ls2[:], ls[:], 0.2, ls[:], ALU.mult, ALU.max)
                    p_self = wk.tile([128, H], F32, tag="p_self")
                    nc.scalar.activation(p_self[:], ls2[:], ACTF.Exp)

                # ---- gathers ----
                g_ps = ps.tile([EPP, 264], F32, tag="gps")
                nc.tensor.matmul(g_ps[:], sblk[:], hAs[:, 0:264],
                                 start=True, stop=True)
                ad_ps = ps.tile([EPP, 16], F32, tag="adrd")
                nc.tensor.matmul(ad_ps[:, 0:8], dblk[:], hAs[:, 264:272],
                                 start=True, stop=True)

                # ---- edge logits ----
                ae = wk.tile([EPP, H], F32, tag="ae")
                nc.vector.tensor_scalar(
                    ae[:], c_web112[:, (li - 1) * H:li * H],
                    c_ea[:, p:p + 1], None, ALU.mult)
                t2 = wk.tile([EPP, H], F32, tag="t2")
                nc.vector.scalar_tensor_tensor(
                    t2[:], g_ps[:, 256:264], 1.0, ae[:], ALU.mult, ALU.add)
                lg = wk.tile([EPP, H], F32, tag="lg")
                nc.vector.scalar_tensor_tensor(
                    lg[:], ad_ps[:, 0:8], 1.0, t2[:], ALU.mult, ALU.add)
                lg2 = wk.tile([EPP, H], F32, tag="lg2")
                nc.vector.scalar_tensor_tensor(
                    lg2[:], lg[:], 0.2, lg[:], ALU.mult, ALU.max)
                p_e = wk.tile([EPP, H], vdt, tag="p_e")
                nc.scalar.activation(p_e[:], lg2[:], ACTF.Exp)

                # ---- denominators ----
                den_ps = ps.tile([128, H], F32, tag="dencnt")
                nc.tensor.matmul(den_ps[:], dtt[:], p_e[:],
                                 start=True, stop=True)
                den_tot = wk.tile([128, H], F32, tag="den_tot")
                if self_loops:
                    nc.vector.tensor_tensor(den_tot[:], den_ps[:], p_self[:],
                                            ALU.add)
                else:
                    nc.vector.tensor_scalar(den_tot[:], den_ps[:], 1e-16, None,
                                            ALU.add)
                rden = wk.tile([128, H], F32, tag="rden")
                nc.vector.reciprocal(rden[:], den_tot[:])
                rden_v = wk.tile([128, H], vdt, tag="rden_v")
                nc.scalar.copy(rden_v[:], rden[:])

                nc.tensor.matmul(ad_ps[:, 8:16], dblk[:], rden_v[:],
                                 start=True, stop=True)

                # ---- messages ----
                pn = wk.tile([EPP, H], vdt, tag="pn")
                nc.vector.tensor_tensor(pn[:], ad_ps[:, 8:16], p_e[:], ALU.mult)
                msg = wk.tile([EPP, HC], vdt, tag="msg")
                nc.vector.tensor_tensor(
                    msg[:].rearrange("p (h c) -> p h c", h=H),
                    g_ps[:, 0:256].rearrange("p (h c) -> p h c", h=H),
                    pn[:, :, None].broadcast_to([EPP, H, HID]), ALU.mult)
                if self_loops:
                    psn = wk.tile([128, H], vdt, tag="psn")
                    nc.vector.tensor_tensor(psn[:], p_self[:], rden[:], ALU.mult)
                    msg_s = wk.tile([128, HC], vdt, tag="msg_s")
                    nc.vector.tensor_tensor(
                        msg_s[:].rearrange("p (h c) -> p h c", h=H),
                        hAs[:, 0:256].rearrange("p (h c) -> p h c", h=H),
                        psn[:, :, None].broadcast_to([128, H, HID]), ALU.mult)

                # ---- scatter (feature-major out) + bias/relu ----
                o_ps = ps.tile([128, 256], F32, tag="outps")
                hT_out = []
                for c in range(2):
                    cs = slice(c * 128, (c + 1) * 128)
                    nc.tensor.matmul(o_ps[:, cs], msg[:, cs], dtt[:],
                                     start=True, stop=not self_loops)
                    if self_loops:
                        nc.tensor.matmul(o_ps[:, cs], msg_s[:, cs], c_id[:, :],
                                         start=False, stop=True)
                    hn = wk.tile([128, 128], vdt, tag=f"hT{c}")
                    nc.scalar.activation(hn[:], o_ps[:, cs], ACTF.Relu,
                                         bias=c_bvec[:, (li - 1) * 2 + c:
                                                     (li - 1) * 2 + c + 1],
                                         scale=1.0)
                    hT_out.append(hn)
                prevT = [t[:, :] for t in hT_out]

            # ---- pooling + MLP ----
            z_ps = ps.tile([128, 16], F32, tag="adrd")
            zg_ps = ps.tile([128, 2], F32, tag="dencnt")
            for c in range(2):
                hT = hT_out[c]
                ge = wk.tile([128, 2], F32, tag=f"ge{c}")
                nc.vector.tensor_reduce(
                    ge[:], hT[:, :].rearrange("p (g n) -> p g n", g=2),
                    mybir.AxisListType.X, ALU.add)
                ge_v = wk.tile([128, 2], vdt, tag=f"gev{c}")
                nc.scalar.copy(ge_v[:], ge[:])
                agent = hT[:, :].rearrange("p (g n) -> p g n", g=2)[:, :, 0:8]
                nc.tensor.matmul(z_ps[:], c_fc1a[:, bass.ts(c, 128)], agent,
                                 start=(c == 0), stop=(c == 1))
                nc.tensor.matmul(zg_ps[:], c_fc1g[:, bass.ts(c, 128)], ge_v[:],
                                 start=(c == 0), stop=(c == 1))
            zgb = wk.tile([128, 2], F32, tag="zgb")
            nc.vector.scalar_tensor_tensor(
                zgb[:], zg_ps[:], 1.0, c_fc1b[:, 0:1].broadcast_to([128, 2]),
                ALU.mult, ALU.add)
            for g in range(2):
                nc.vector.tensor_scalar(
                    zbat[:, po * 16 + g * 8: po * 16 + (g + 1) * 8],
                    z_ps[:, g * 8:(g + 1) * 8], zgb[:, g:g + 1], 0.0,
                    ALU.add, ALU.max)

            if po == 7 or p == NP - 1:
                oc_ps = ps.tile([OUT, 128], F32, tag="transp")
                nc.tensor.matmul(oc_ps[:], c_fc2w[:, :], zbat[:],
                                 start=True, stop=True)
                oct_i = p // 8
                nvalid = (po + 1) * 16
                nc.vector.tensor_scalar(
                    out_acc[:, oct_i * 128:oct_i * 128 + nvalid],
                    oc_ps[:, 0:nvalid], c_fc2b[:, 0:1], None, ALU.add)

        nc.sync.dma_start(out_d[:, :], out_acc[:])

    nc.compile()
    return nc


# ---------------- host-side packing ----------------

def _np_vdt(vdt):
    import ml_dtypes
    return {mybir.dt.bfloat16: ml_dtypes.bfloat16,
            mybir.dt.float32: np.float32}[vdt]


def host_prep(inputs, npairs=GPC // 2, vdt=mybir.dt.bfloat16):
    """Returns list of per-core input dicts."""
    nv = _np_vdt(vdt)
    x = np.asarray(inputs["x"], np.float32)
    ei = np.asarray(inputs["edge_index"])
    eattr = np.asarray(inputs["edge_attr"], np.float32)

    def pack_w(l):
        W = np.asarray(inputs[f"W{l}"], np.float32)
        a_s = np.asarray(inputs[f"as{l}"], np.float32)
        a_d = np.asarray(inputs[f"ad{l}"], np.float32)
        Ps = np.einsum("fkc,kc->fk", W.reshape(W.shape[0], H, HID), a_s)
        Pd = np.einsum("fkc,kc->fk", W.reshape(W.shape[0], H, HID), a_d)
        return np.concatenate([W, Ps, Pd, Ps + Pd], axis=1).astype(nv)

    def w_e(l):
        We = np.asarray(inputs[f"We{l}"], np.float32).reshape(H, HID)
        a_e = np.asarray(inputs[f"ae{l}"], np.float32)
        return (We * a_e).sum(-1)   # [H]

    waug = {l: pack_w(l) for l in (1, 2, 3)}
    for l in (2, 3):
        waug[l] = np.concatenate([waug[l][:128], waug[l][128:]], axis=1)
    wev = np.concatenate([w_e(l) for l in (1, 2, 3)])          # [24]
    web112 = np.broadcast_to(wev, (EPP, 24)).astype(np.float32).copy()
    web128 = np.broadcast_to(wev, (128, 24)).astype(np.float32).copy()
    bvec = np.stack([np.asarray(inputs[f"b{l}"], np.float32)
                     .reshape(2, 128)[c]
                     for l in (1, 2, 3) for c in range(2)], axis=1)  # [128,6]
    fc1_w = np.asarray(inputs["fc1_w"], np.float32)
    fc1a = np.concatenate([fc1_w[:128], fc1_w[128:HC]], axis=1).astype(nv)
    fc1g = np.concatenate([fc1_w[HC:HC + 128] / P,
                           fc1_w[HC + 128:] / P], axis=1).astype(nv)
    fc1b = np.asarray(inputs["fc1_b"], np.float32).reshape(128, 1)
    fc2w = np.asarray(inputs["fc2_w"], np.float32).astype(nv)
    fc2b = np.asarray(inputs["fc2_b"], np.float32).reshape(OUT, 1)
    identm = np.eye(128, dtype=np.float32).astype(nv)
    iota = np.broadcast_to(np.arange(128, dtype=np.float32),
                           (EPP, 128)).copy()
    ones1 = np.ones((EPP, 1), np.float32).astype(nv)

    maps = []
    npc = GPC * P               # nodes per core
    epc = GPC * OBS             # edges per core
    for m in range(NCORES):
        nsl = slice(m * npc, (m + 1) * npc)
        esl = slice(m * epc, (m + 1) * epc)
        xt = np.ascontiguousarray(x[nsl].T).astype(nv)          # [16, npc]
        src = np.asarray(ei[0][esl], np.int64) - m * npc
        dst = np.asarray(ei[1][esl], np.int64) - m * npc
        # localize to pair: pair p covers nodes [p*128, (p+1)*128)
        pairs = np.arange(GPC // 2).repeat(EPP)
        src_l = (src.reshape(-1) - pairs * 128).astype(np.float32)
        dst_l = (dst.reshape(-1) - pairs * 128).astype(np.float32)
        esrc = np.ascontiguousarray(src_l.reshape(-1, EPP).T)    # [112, NPall]
        edst = np.ascontiguousarray(dst_l.reshape(-1, EPP).T)
        eat = np.ascontiguousarray(
            eattr[esl].reshape(-1, EPP).T).astype(np.float32)
        maps.append({
            "xt": xt[:, :npairs * 128],
            "esrc": esrc[:, :npairs], "edst": edst[:, :npairs],
            "eattr": eat[:, :npairs],
            "eattr_v": eat[:, :npairs].astype(nv),
            "waug1": waug[1], "waug2": waug[2], "waug3": waug[3],
            "web112": web112, "web128": web128, "bvec": bvec,
            "fc1a": fc1a, "fc1g": fc1g, "fc1b": fc1b,
            "fc2w": fc2w, "fc2b": fc2b,
            "ident": identm, "iota": iota, "ones1": ones1,
        })
    return maps


def unpack_out(res_list, npairs=GPC // 2):
    outs = []
    for m in range(NCORES):
        o = res_list[m]["out"]                       # [2, NP*16]
        o = o.reshape(OUT, npairs, 2, A).transpose(1, 2, 3, 0)
        outs.append(o.reshape(npairs * 2, A, OUT))
    return np.concatenate(outs, axis=0).astype(np.float32)


# ---------------- entry point ----------------

LAST_EXEC_NS = None
_NC_CACHE = {}


def kernel(**inputs) -> np.ndarray:
    """Full-input GAT forward on 8 NeuronCores; returns [4096, 8, 2] f32."""
    global LAST_EXEC_NS
    import os
    vdt = mybir.dt.bfloat16
    npairs = GPC // 2
    key = (npairs, vdt)
    if key not in _NC_CACHE:
        _NC_CACHE[key] = build(npairs, vdt=vdt, num_devices=NCORES)
    nc = _NC_CACHE[key]
    maps = host_prep(inputs, npairs=npairs, vdt=vdt)
    trace = os.environ.get("BASS_GAT_TRACE") == "1"
    res = run_bass_kernel_spmd(nc, maps, core_ids=list(range(NCORES)),
                               trace=trace, trace_cores=[0] if trace else None)
    LAST_EXEC_NS = res.exec_time_ns
    return unpack_out([r for r in res.results], npairs=npairs)


# revision 3
# speedup vs baseline: 1.4391x; 1.4391x over previous
"""GAT model Bass/Tile kernel for TRN2 (self-contained, octet-batched).

Per core: 512 graphs as 256 pairs (128 nodes / 112 edges). Pairs are
processed in octets (8 pairs): per-edge/per-node attention scalars are
batched into [*, 64] ops across the octet; fat value ops run at duet
(2-pair) granularity; engines are balanced DVE/ACT/GPSIMD/PE.
"""
import numpy as np
from contextlib import ExitStack

import concourse.bass as bass
import concourse.tile as tile
from concourse import bacc, mybir
from concourse.bass_utils import run_bass_kernel_spmd

F32 = mybir.dt.float32
I32 = mybir.dt.int32

B, A, OBS = 4096, 8, 56
P = 64
H, HID, HC = 8, 32, 256
IN, OUT = 16, 2
NCORES = 8
GPC = B // NCORES
EPP = 2 * OBS
ALU = mybir.AluOpType
ACTF = mybir.ActivationFunctionType

# small_ps column regions (f32)
AS_, AD_, DEN_, RD_, CNT_, ZG_, Z_ = 0, 64, 128, 192, 256, 272, 288


def build(npairs: int, vdt=mybir.dt.bfloat16, num_devices: int = NCORES):
    assert npairs % 8 == 0
    nc = bacc.Bacc("TRN2", target_bir_lowering=False, debug=False,
                   num_devices=num_devices)
    NP = npairs

    def din(name, shape, dt):
        return nc.dram_tensor(name, shape, dt, kind="ExternalInput").ap()

    xt = din("xt", [IN, NP * 128], vdt)
    esrc = din("esrc", [EPP, NP], F32)
    edst = din("edst", [EPP, NP], F32)
    eattr = din("eattr", [EPP, NP], F32)
    eap = din("eap", [EPP, 2 * NP], vdt)
    waug1 = din("waug1", [IN, 272], vdt)
    waug2 = din("waug2", [128, 544], vdt)
    waug3 = din("waug3", [128, 544], vdt)
    webe = din("webe", [EPP, 3 * 64], F32)    # w_e tiled 8x per layer
    webn = din("webn", [128, 3 * 64], F32)
    fc1a = din("fc1a", [128, HC], vdt)
    fc1g = din("fc1g", [128, HC], vdt)
    fc1b = din("fc1b", [128, 1], F32)
    fc2w = din("fc2w", [128, OUT], vdt)
    fc2b = din("fc2b", [OUT, 1], F32)
    ident = din("ident", [128, 128], vdt)
    iota = din("iota", [EPP, 128], F32)

    out_d = nc.dram_tensor("out", [OUT, NP * 16], F32, kind="ExternalOutput").ap()

    with tile.TileContext(nc) as tc, ExitStack() as ctx:
        cpool = ctx.enter_context(tc.tile_pool(name="const", bufs=1))
        wk = ctx.enter_context(tc.tile_pool(name="work", bufs=2))
        eb = ctx.enter_context(tc.tile_pool(name="edges", bufs=18))
        ps = ctx.enter_context(tc.tile_pool(name="psum", bufs=1, space="PSUM"))

        def cload(ap, tag):
            t = cpool.tile(list(ap.shape), ap.dtype, tag=tag)
            nc.sync.dma_start(t[:], ap[:, :])
            return t

        c_w1, c_w2, c_w3 = cload(waug1, "w1"), cload(waug2, "w2"), cload(waug3, "w3")
        c_webe, c_webn = cload(webe, "webe"), cload(webn, "webn")
        c_fc1a, c_fc1g = cload(fc1a, "fc1a"), cload(fc1g, "fc1g")
        c_fc1b, c_fc2w, c_fc2b = cload(fc1b, "fc1b"), cload(fc2w, "fc2w"), cload(fc2b, "fc2b")
        c_id, c_iota = cload(ident, "ident"), cload(iota, "iota")
        c_esrc, c_edst = cload(esrc, "esrc"), cload(edst, "edst")
        c_ea, c_eap = cload(eattr, "eattr"), cload(eap, "eap")

        out_acc = cpool.tile([OUT, NP * 16], F32, tag="out_acc")

        wchunks = {1: [c_w1[:, :]],
                   2: [c_w2[:, 0:272], c_w2[:, 272:544]],
                   3: [c_w3[:, 0:272], c_w3[:, 272:544]]}

        for oct_i in range(NP // 8):
            p0 = oct_i * 8
            sp = ps.tile([128, 416], F32, tag="small")
            hAso = wk.tile([128, 8 * 272], vdt, tag="hAso")
            hAv = hAso[:, :].rearrange("p (pr x) -> p pr x", pr=8)

            # ---- phase A: edge structure + x loads (per pair) ----
            sblk_l, dblk_l, dtt_l, x0_l = [], [], [], []
            for j in range(8):
                pp = p0 + j
                st = eb.tile([EPP, 128], vdt, tag="st")
                nc.vector.tensor_scalar(st[:], c_iota[:], c_esrc[:, pp:pp + 1],
                                        None, ALU.is_equal)
                dtt = eb.tile([EPP, 128], vdt, tag="dtt")
                nc.vector.tensor_scalar(dtt[:], c_iota[:], c_edst[:, pp:pp + 1],
                                        None, ALU.is_equal)
                tr_ps = ps.tile([128, 224], vdt, tag="transp")
                nc.tensor.transpose(tr_ps[:, 0:112], st[:], c_id[0:EPP, 0:EPP])
                nc.tensor.transpose(tr_ps[:, 112:224], dtt[:], c_id[0:EPP, 0:EPP])
                sblk = eb.tile([128, EPP], vdt, tag="sblk")
                nc.vector.tensor_copy(sblk[:], tr_ps[:, 0:112])
                dblk = eb.tile([128, EPP], vdt, tag="dblk")
                nc.scalar.copy(dblk[:], tr_ps[:, 112:224])
                x0 = eb.tile([IN, 128], vdt, tag="x0")
                nc.sync.dma_start(x0[:], xt[:, pp * 128:(pp + 1) * 128])
                sblk_l.append(sblk); dblk_l.append(dblk)
                dtt_l.append(dtt); x0_l.append(x0)

            prevT = [[x0_l[j][:, :]] for j in range(8)]
            ea_loop = None
            hT_l3 = None

            for li in (1, 2, 3):
                wch = wchunks[li]
                self_loops = li > 1

                # ---- MM1 per duet + hAs copy + small gathers ----
                for d in range(4):
                    h2 = ps.tile([128, 1024], F32, tag="h2")
                    for jj in range(2):
                        j = 2 * d + jj
                        hs = h2[:, jj * 512: jj * 512 + 272]
                        for kc, (sta, wc) in enumerate(zip(prevT[j], wch)):
                            nc.tensor.matmul(hs, sta, wc, start=(kc == 0),
                                             stop=(kc == len(wch) - 1))
                    nc.vector.tensor_copy(
                        hAso[:, d * 544:(d + 1) * 544]
                        .rearrange("p (a x) -> p a x", a=2),
                        h2[:, :].rearrange("p (a x) -> p a x", a=2)[:, :, 0:272])
                for j in range(8):
                    nc.tensor.matmul(sp[0:112, AS_ + j * 8:AS_ + j * 8 + 8],
                                     sblk_l[j][:], hAv[:, j, 256:264],
                                     start=True, stop=True)
                    nc.tensor.matmul(sp[0:112, AD_ + j * 8:AD_ + j * 8 + 8],
                                     dblk_l[j][:], hAv[:, j, 264:272],
                                     start=True, stop=True)
                    if li == 1:
                        nc.tensor.matmul(
                            sp[:, CNT_ + j * 2:CNT_ + j * 2 + 2], dtt_l[j][:],
                            c_eap[:, 2 * (p0 + j):2 * (p0 + j) + 2],
                            start=True, stop=True)

                if li == 1:
                    cntv = sp[:, CNT_:CNT_ + 16].rearrange(
                        "p (pr two) -> p pr two", two=2)
                    cntm = wk.tile([128, 8], F32, tag="cntm")
                    nc.vector.tensor_scalar(cntm[:], cntv[:, :, 1:2], 1.0,
                                            None, ALU.max)
                    rc = wk.tile([128, 8], F32, tag="rc")
                    nc.vector.reciprocal(rc[:], cntm[:])
                    ea_loop = wk.tile([128, 8], F32, tag="ea_loop")
                    nc.vector.tensor_tensor(ea_loop[:], cntv[:, :, 0:1]
                                            .rearrange("p a b -> p (a b)"),
                                            rc[:], ALU.mult)

                # ---- batched edge logits ----
                ae = wk.tile([EPP, 64], F32, tag="ae")
                nc.gpsimd.tensor_tensor(
                    ae[:].rearrange("p (a h) -> p a h", a=8),
                    c_webe[:, (li - 1) * 64:li * 64]
                    .rearrange("p (a h) -> p a h", a=8),
                    c_ea[:, p0:p0 + 8][:, :, None].broadcast_to([EPP, 8, H]),
                    ALU.mult)
                t_o = wk.tile([EPP, 64], F32, tag="t_o")
                nc.vector.scalar_tensor_tensor(
                    t_o[:], sp[0:112, AS_:AS_ + 64], 1.0, ae[:],
                    ALU.mult, ALU.add)
                lg = wk.tile([EPP, 64], F32, tag="lg")
                nc.vector.scalar_tensor_tensor(
                    lg[:], sp[0:112, AD_:AD_ + 64], 1.0, t_o[:],
                    ALU.mult, ALU.add)
                lg2 = wk.tile([EPP, 64], F32, tag="lg2")
                nc.vector.scalar_tensor_tensor(
                    lg2[:], lg[:], 0.2, lg[:], ALU.mult, ALU.max)
                p_e = wk.tile([EPP, 64], vdt, tag="p_e")
                nc.scalar.activation(p_e[:], lg2[:], ACTF.Exp)

                p_self = None
                if self_loops:
                    sae = wk.tile([128, 64], F32, tag="sae")
                    nc.gpsimd.tensor_tensor(
                        sae[:].rearrange("p (a h) -> p a h", a=8),
                        c_webn[:, (li - 1) * 64:li * 64]
                        .rearrange("p (a h) -> p a h", a=8),
                        ea_loop[:][:, :, None].broadcast_to([128, 8, H]),
                        ALU.mult)
                    s1 = wk.tile([128, 64], F32, tag="s1")
                    nc.gpsimd.tensor_tensor(
                        s1[:].rearrange("p (a h) -> p a h", a=8),
                        hAv[:, :, 256:264],
                        hAv[:, :, 264:272], ALU.add)
                    s2 = wk.tile([128, 64], F32, tag="s2")
                    nc.gpsimd.tensor_tensor(
                        s2[:], s1[:], sae[:], ALU.add)
                    s3 = wk.tile([128, 64], F32, tag="s3")
                    nc.vector.scalar_tensor_tensor(
                        s3[:], s2[:], 0.2, s2[:], ALU.mult, ALU.max)
                    p_self = wk.tile([128, 64], F32, tag="p_self")
                    nc.scalar.activation(p_self[:], s3[:], ACTF.Exp)

                # ---- denominators ----
                for j in range(8):
                    nc.tensor.matmul(sp[:, DEN_ + j * 8:DEN_ + j * 8 + 8],
                                     dtt_l[j][:], p_e[:, j * 8:j * 8 + 8],
                                     start=True, stop=True)
                dtot = wk.tile([128, 64], F32, tag="dtot")
                if self_loops:
                    nc.vector.tensor_tensor(dtot[:], sp[:, DEN_:DEN_ + 64],
                                            p_self[:], ALU.add)
                else:
                    nc.vector.tensor_scalar(dtot[:], sp[:, DEN_:DEN_ + 64],
                                            1e-16, None, ALU.add)
                rden = wk.tile([128, 64], F32, tag="rden")
                nc.vector.reciprocal(rden[:], dtot[:])
                rden_v = wk.tile([128, 64], vdt, tag="rden_v")
                nc.scalar.copy(rden_v[:], rden[:])
                for j in range(8):
                    nc.tensor.matmul(sp[0:112, RD_ + j * 8:RD_ + j * 8 + 8],
                                     dblk_l[j][:], rden_v[:, j * 8:j * 8 + 8],
                                     start=True, stop=True)
                pn = wk.tile([EPP, 64], vdt, tag="pn")
                nc.vector.tensor_tensor(pn[:], sp[0:112, RD_:RD_ + 64], p_e[:],
                                        ALU.mult)
                if self_loops:
                    psn = wk.tile([128, 64], vdt, tag="psn")
                    nc.gpsimd.tensor_tensor(psn[:], p_self[:], rden[:], ALU.mult)

                # ---- phase B per duet: gather h, messages, scatter, relu ----
                hT_new = []
                for d in range(4):
                    g2 = ps.tile([EPP, 1024], F32, tag="g2")
                    for jj in range(2):
                        j = 2 * d + jj
                        nc.tensor.matmul(g2[:, jj * 512:jj * 512 + 256],
                                         sblk_l[j][:], hAv[:, j, 0:256],
                                         start=True, stop=True)
                    msg2 = wk.tile([EPP, 512], vdt, tag="msg2")
                    nc.vector.tensor_tensor(
                        msg2[:].rearrange("p (a h c) -> p a h c", a=2, h=H),
                        g2[:, :].rearrange("p (a x) -> p a x", a=2)[:, :, 0:256]
                        .rearrange("p a (h c) -> p a h c", h=H),
                        pn[:, d * 16:(d + 1) * 16]
                        .rearrange("p (a h) -> p a h", a=2)[:, :, :, None]
                        .broadcast_to([EPP, 2, H, HID]), ALU.mult)
                    if self_loops:
                        msgs2 = wk.tile([128, 512], vdt, tag="msgs2")
                        for jj in range(2):
                            j = 2 * d + jj
                            nc.gpsimd.tensor_tensor(
                                msgs2[:, jj * 256:(jj + 1) * 256]
                                .rearrange("p (h c) -> p h c", h=H),
                                hAv[:, j, 0:256]
                                .rearrange("p (h c) -> p h c", h=H),
                                psn[:, j * 8:(j + 1) * 8][:, :, None]
                                .broadcast_to([128, H, HID]), ALU.mult)
                    o2 = ps.tile([128, 512], F32, tag="out2")
                    for jj in range(2):
                        j = 2 * d + jj
                        for c in range(2):
                            cs = slice(jj * 256 + c * 128, jj * 256 + c * 128 + 128)
                            nc.tensor.matmul(
                                o2[:, cs], msg2[:, jj * 256 + c * 128:
                                                jj * 256 + (c + 1) * 128],
                                dtt_l[j][:], start=True, stop=not self_loops)
                            if self_loops:
                                nc.tensor.matmul(
                                    o2[:, cs], msgs2[:, jj * 256 + c * 128:
                                                     jj * 256 + (c + 1) * 128],
                                    c_id[:, :], start=False, stop=True)
                    hT2 = wk.tile([128, 512], vdt, tag=f"hT{li}_{d}")
                    nc.scalar.activation(hT2[:], o2[:], ACTF.Relu)
                    hT_new.append(hT2)
                    for jj in range(2):
                        j = 2 * d + jj
                        prevT[j] = [hT2[:, jj * 256:jj * 256 + 128],
                                    hT2[:, jj * 256 + 128:jj * 256 + 256]]
                hT_l3 = hT_new

            # ---- pooling + MLP ----
            gev_l = []
            for d in range(4):
                ge = wk.tile([128, 8], F32, tag=f"ge{d}")
                nc.vector.tensor_reduce(
                    ge[:], hT_l3[d][:, :].rearrange(
                        "p (a b g n) -> p a b g n", a=2, b=2, g=2),
                    mybir.AxisListType.X, ALU.add)
                gev = wk.tile([128, 8], vdt, tag=f"gev{d}")
                nc.scalar.copy(gev[:], ge[:])
                gev_l.append(gev)
            for j in range(8):
                d, jj = j // 2, j % 2
                for c in range(2):
                    agent = hT_l3[d][:, jj * 256 + c * 128:
                                     jj * 256 + (c + 1) * 128] \
                        .rearrange("p (g n) -> p g n", g=2)[:, :, 0:8]
                    nc.tensor.matmul(sp[:, Z_ + j * 16:Z_ + (j + 1) * 16],
                                     c_fc1a[:, bass.ts(c, 128)], agent,
                                     start=(c == 0), stop=(c == 1))
                for c in range(2):
                    nc.tensor.matmul(sp[:, ZG_ + j * 2:ZG_ + (j + 1) * 2],
                                     c_fc1g[:, bass.ts(c, 128)],
                                     gev_l[d][:, jj * 4 + c * 2:jj * 4 + c * 2 + 2],
                                     start=(c == 0), stop=(c == 1))
            zgb = wk.tile([128, 16], F32, tag="zgb")
            nc.vector.scalar_tensor_tensor(
                zgb[:], sp[:, ZG_:ZG_ + 16], 1.0,
                c_fc1b[:, 0:1].broadcast_to([128, 16]), ALU.mult, ALU.add)
            zt = wk.tile([128, 128], F32, tag="zt")
            nc.vector.scalar_tensor_tensor(
                zt[:].rearrange("p (a b) -> p a b", a=16),
                sp[:, Z_:Z_ + 128].rearrange("p (a b) -> p a b", a=16), 1.0,
                zgb[:][:, :, None].broadcast_to([128, 16, 8]),
                ALU.mult, ALU.add)
            zbat = wk.tile([128, 128], vdt, tag="zbat")
            nc.scalar.activation(zbat[:], zt[:], ACTF.Relu)
            oc_ps = ps.tile([OUT, 128], F32, tag="transp")
            nc.tensor.matmul(oc_ps[:], c_fc2w[:, :], zbat[:],
                             start=True, stop=True)
            nc.vector.tensor_scalar(out_acc[:, oct_i * 128:(oct_i + 1) * 128],
                                    oc_ps[:], c_fc2b[:, 0:1], None, ALU.add)

        nc.sync.dma_start(out_d[:, :], out_acc[:])

    nc.compile()
    return nc


# ---------------- host-side packing ----------------

def _np_vdt(vdt):
    import ml_dtypes
    return {mybir.dt.bfloat16: ml_dtypes.bfloat16,
            mybir.dt.float32: np.float32}[vdt]


def host_prep(inputs, npairs=GPC // 2, vdt=mybir.dt.bfloat16):
    nv = _np_vdt(vdt)
    x = np.asarray(inputs["x"], np.float32)
    ei = np.asarray(inputs["edge_index"])
    eattr = np.asarray(inputs["edge_attr"], np.float32)
    for l in (1, 2, 3):
        assert not np.any(np.asarray(inputs[f"b{l}"])), "GAT bias must be 0"

    def pack_w(l):
        W = np.asarray(inputs[f"W{l}"], np.float32)
        a_s = np.asarray(inputs[f"as{l}"], np.float32)
        a_d = np.asarray(inputs[f"ad{l}"], np.float32)
        Ps = np.einsum("fkc,kc->fk", W.reshape(W.shape[0], H, HID), a_s)
        Pd = np.einsum("fkc,kc->fk", W.reshape(W.shape[0], H, HID), a_d)
        return np.concatenate([W, Ps, Pd], axis=1).astype(nv)

    def w_e(l):
        We = np.asarray(inputs[f"We{l}"], np.float32).reshape(H, HID)
        a_e = np.asarray(inputs[f"ae{l}"], np.float32)
        return (We * a_e).sum(-1)

    waug = {l: pack_w(l) for l in (1, 2, 3)}
    for l in (2, 3):
        waug[l] = np.concatenate([waug[l][:128], waug[l][128:]], axis=1)
    wev = np.concatenate([np.tile(w_e(l), 8) for l in (1, 2, 3)])   # [192]
    webe = np.broadcast_to(wev, (EPP, 192)).astype(np.float32).copy()
    webn = np.broadcast_to(wev, (128, 192)).astype(np.float32).copy()
    fc1_w = np.asarray(inputs["fc1_w"], np.float32)
    fc1a = np.concatenate([fc1_w[:128], fc1_w[128:HC]], axis=1).astype(nv)
    fc1g = np.concatenate([fc1_w[HC:HC + 128] / P,
                           fc1_w[HC + 128:] / P], axis=1).astype(nv)
    fc1b = np.asarray(inputs["fc1_b"], np.float32).reshape(128, 1)
    fc2w = np.asarray(inputs["fc2_w"], np.float32).astype(nv)
    fc2b = np.asarray(inputs["fc2_b"], np.float32).reshape(OUT, 1)
    identm = np.eye(128, dtype=np.float32).astype(nv)
    iota = np.broadcast_to(np.arange(128, dtype=np.float32), (EPP, 128)).copy()

    maps = []
    npc = GPC * P
    epc = GPC * OBS
    for m in range(NCORES):
        nsl = slice(m * npc, (m + 1) * npc)
        esl = slice(m * epc, (m + 1) * epc)
        xt = np.ascontiguousarray(x[nsl].T).astype(nv)
        src = np.asarray(ei[0][esl], np.int64) - m * npc
        dst = np.asarray(ei[1][esl], np.int64) - m * npc
        pairs = np.arange(GPC // 2).repeat(EPP)
        src_l = (src.reshape(-1) - pairs * 128).astype(np.float32)
        dst_l = (dst.reshape(-1) - pairs * 128).astype(np.float32)
        esrc = np.ascontiguousarray(src_l.reshape(-1, EPP).T)
        edst = np.ascontiguousarray(dst_l.reshape(-1, EPP).T)
        eat = np.ascontiguousarray(eattr[esl].reshape(-1, EPP).T).astype(np.float32)
        eap_arr = np.empty((EPP, 2 * npairs), np.float32)
        eap_arr[:, 0::2] = eat[:, :npairs]
        eap_arr[:, 1::2] = 1.0
        maps.append({
            "xt": xt[:, :npairs * 128],
            "esrc": esrc[:, :npairs], "edst": edst[:, :npairs],
            "eattr": eat[:, :npairs], "eap": eap_arr.astype(nv),
            "waug1": waug[1], "waug2": waug[2], "waug3": waug[3],
            "webe": webe, "webn": webn,
            "fc1a": fc1a, "fc1g": fc1g, "fc1b": fc1b,
            "fc2w": fc2w, "fc2b": fc2b,
            "ident": identm, "iota": iota,
        })
    return maps


def unpack_out(res_list, npairs=GPC // 2):
    outs = []
    for m in range(NCORES):
        o = res_list[m]["out"]
        o = o.reshape(OUT, npairs, 2, A).transpose(1, 2, 3, 0)
        outs.append(o.reshape(npairs * 2, A, OUT))
    return np.concatenate(outs, axis=0).astype(np.float32)


# ---------------- entry point ----------------

LAST_EXEC_NS = None
_NC_CACHE = {}


def kernel(**inputs) -> np.ndarray:
    """Full-input GAT forward on 8 NeuronCores; returns [4096, 8, 2] f32."""
    global LAST_EXEC_NS
    import os
    vdt = mybir.dt.bfloat16
    npairs = GPC // 2
    key = (npairs, vdt)
    if key not in _NC_CACHE:
        _NC_CACHE[key] = build(npairs, vdt=vdt, num_devices=NCORES)
    nc = _NC_CACHE[key]
    maps = host_prep(inputs, npairs=npairs, vdt=vdt)
    trace = os.environ.get("BASS_GAT_TRACE") == "1"
    res = None
    for attempt in range(3):
        try:
            res = run_bass_kernel_spmd(
                nc, maps, core_ids=list(range(NCORES)),
                trace=trace and attempt == 0,
                trace_cores=[0] if trace and attempt == 0 else None)
            break
        except Exception:
            if attempt == 2:
                raise
            import time
            time.sleep(5)
    LAST_EXEC_NS = res.exec_time_ns
    return unpack_out([r for r in res.results], npairs=npairs)


# revision 4
# speedup vs baseline: 1.4895x; 1.0351x over previous
"""GAT model Bass/Tile kernel for TRN2 (self-contained, octet-batched).

Per core: 512 graphs as 256 pairs (128 nodes / 112 edges). Pairs are
processed in octets (8 pairs): per-edge/per-node attention scalars are
batched into [*, 64] ops across the octet; fat value ops run at duet
(2-pair) granularity; engines are balanced DVE/ACT/GPSIMD/PE.
"""
import numpy as np
from contextlib import ExitStack

import concourse.bass as bass
import concourse.tile as tile
from concourse import bacc, mybir
from concourse.bass_utils import run_bass_kernel_spmd

F32 = mybir.dt.float32
I32 = mybir.dt.int32

B, A, OBS = 4096, 8, 56
P = 64
H, HID, HC = 8, 32, 256
IN, OUT = 16, 2
NCORES = 8
GPC = B // NCORES
EPP = 2 * OBS
ALU = mybir.AluOpType
ACTF = mybir.ActivationFunctionType

# small_ps column regions (f32); Z/ZG/oc reuse the same tile post-L3
ASD_, DEN_, RD_, CNT_, Z_, ZG_ = 0, 64, 128, 192, 0, 128


def build(npairs: int, vdt=mybir.dt.bfloat16, num_devices: int = NCORES):
    assert npairs % 8 == 0
    nc = bacc.Bacc("TRN2", target_bir_lowering=False, debug=False,
                   num_devices=num_devices)
    NP = npairs

    def din(name, shape, dt):
        return nc.dram_tensor(name, shape, dt, kind="ExternalInput").ap()

    xt = din("xt", [IN, NP * 128], vdt)
    esrcb = din("esrcb", [NP, EPP], F32)
    edstb = din("edstb", [NP, EPP], F32)
    edst = din("edst", [EPP, NP], F32)
    eattr = din("eattr", [EPP, NP], F32)
    eap = din("eap", [EPP, 2 * NP], vdt)
    waug1 = din("waug1", [IN, 272], vdt)
    waug2 = din("waug2", [128, 544], vdt)
    waug3 = din("waug3", [128, 544], vdt)
    webe = din("webe", [EPP, 3 * 64], F32)    # w_e tiled 8x per layer
    webn = din("webn", [128, 3 * 64], F32)
    fc1a = din("fc1a", [128, HC], vdt)
    fc1g = din("fc1g", [128, HC], vdt)
    fc1b = din("fc1b", [128, 1], F32)
    fc2w = din("fc2w", [128, OUT], vdt)
    fc2b = din("fc2b", [OUT, 1], F32)
    ident = din("ident", [128, 128], vdt)
    iota = din("iota", [EPP, 128], F32)
    iotac = din("iotac", [128, 1], F32)

    out_d = nc.dram_tensor("out", [OUT, NP * 16], F32, kind="ExternalOutput").ap()

    with tile.TileContext(nc) as tc, ExitStack() as ctx:
        cpool = ctx.enter_context(tc.tile_pool(name="const", bufs=1))
        wk = ctx.enter_context(tc.tile_pool(name="work", bufs=4))
        eb = ctx.enter_context(tc.tile_pool(name="edges", bufs=24))
        ps = ctx.enter_context(tc.tile_pool(name="psum", bufs=1, space="PSUM"))

        def cload(ap, tag):
            t = cpool.tile(list(ap.shape), ap.dtype, tag=tag)
            nc.sync.dma_start(t[:], ap[:, :])
            return t

        c_w1, c_w2, c_w3 = cload(waug1, "w1"), cload(waug2, "w2"), cload(waug3, "w3")
        c_webe, c_webn = cload(webe, "webe"), cload(webn, "webn")
        c_fc1a, c_fc1g = cload(fc1a, "fc1a"), cload(fc1g, "fc1g")
        c_fc1b, c_fc2w, c_fc2b = cload(fc1b, "fc1b"), cload(fc2w, "fc2w"), cload(fc2b, "fc2b")
        c_id, c_iota = cload(ident, "ident"), cload(iota, "iota")
        c_iotac = cload(iotac, "iotac")
        c_edst = cload(edst, "edst")
        c_ea, c_eap = cload(eattr, "eattr"), cload(eap, "eap")

        out_acc = cpool.tile([OUT, NP * 16], F32, tag="out_acc")

        wchunks = {1: [c_w1[:, :]],
                   2: [c_w2[:, 0:272], c_w2[:, 272:544]],
                   3: [c_w3[:, 0:272], c_w3[:, 272:544]]}

        for oct_i in range(NP // 8):
            p0 = oct_i * 8
            sp = ps.tile([128, 208], F32, tag="small", bufs=2)
            hAso = wk.tile([128, 8 * 272], vdt, tag="hAso")
            hAv = hAso[:, :].rearrange("p (pr x) -> p pr x", pr=8)

            # ---- phase A: edge structure + x loads ----
            srcb = eb.tile([128, 8 * EPP], F32, tag="srcb", bufs=2)
            nc.sync.dma_start(srcb[:], esrcb[p0:p0 + 8, :]
                              .rearrange("a b -> (a b)")[None, :]
                              .broadcast_to([128, 8 * EPP]))
            dstb = eb.tile([128, 8 * EPP], F32, tag="dstb", bufs=2)
            nc.sync.dma_start(dstb[:], edstb[p0:p0 + 8, :]
                              .rearrange("a b -> (a b)")[None, :]
                              .broadcast_to([128, 8 * EPP]))
            sblk_o = eb.tile([128, 8 * EPP], vdt, tag="sblk_o", bufs=2)
            nc.vector.tensor_scalar(sblk_o[:], srcb[:], c_iotac[:, 0:1],
                                    None, ALU.is_equal)
            dblk_o = eb.tile([128, 8 * EPP], vdt, tag="dblk_o", bufs=2)
            nc.vector.tensor_scalar(dblk_o[:], dstb[:], c_iotac[:, 0:1],
                                    None, ALU.is_equal)
            sblk_l = [sblk_o[:, j * EPP:(j + 1) * EPP] for j in range(8)]
            dblk_l = [dblk_o[:, j * EPP:(j + 1) * EPP] for j in range(8)]
            dtt_l, x0_l = [], []
            for j in range(8):
                pp = p0 + j
                dtt = eb.tile([EPP, 128], vdt, tag="dtt")
                nc.vector.tensor_scalar(dtt[:], c_iota[:], c_edst[:, pp:pp + 1],
                                        None, ALU.is_equal)
                x0 = eb.tile([IN, 128], vdt, tag="x0")
                nc.sync.dma_start(x0[:], xt[:, pp * 128:(pp + 1) * 128])
                dtt_l.append(dtt); x0_l.append(x0)

            prevT = [[x0_l[j][:, :]] for j in range(8)]
            ea_loop = None
            hT_l3 = None

            for li in (1, 2, 3):
                wch = wchunks[li]
                self_loops = li > 1

                # ---- MM1 per pair + hAs copy + small gathers ----
                for j in range(8):
                    h2 = ps.tile([128, 272], F32, tag="h2", bufs=2)
                    for kc, (sta, wc) in enumerate(zip(prevT[j], wch)):
                        nc.tensor.matmul(h2[:], sta, wc, start=(kc == 0),
                                         stop=(kc == len(wch) - 1))
                    nc.vector.tensor_copy(hAso[:, j * 272:(j + 1) * 272], h2[:])
                    nc.tensor.matmul(sp[0:112, ASD_ + j * 8:ASD_ + j * 8 + 8],
                                     sblk_l[j], hAv[:, j, 256:264],
                                     start=True, stop=False)
                    nc.tensor.matmul(sp[0:112, ASD_ + j * 8:ASD_ + j * 8 + 8],
                                     dblk_l[j], hAv[:, j, 264:272],
                                     start=False, stop=True)
                    if li == 1:
                        nc.tensor.matmul(
                            sp[:, CNT_ + j * 2:CNT_ + j * 2 + 2], dtt_l[j][:],
                            c_eap[:, 2 * (p0 + j):2 * (p0 + j) + 2],
                            start=True, stop=True)

                if li == 1:
                    cntv = sp[:, CNT_:CNT_ + 16].rearrange(
                        "p (pr two) -> p pr two", two=2)
                    cntm = wk.tile([128, 8], F32, tag="cntm")
                    nc.vector.tensor_scalar(cntm[:], cntv[:, :, 1:2], 1.0,
                                            None, ALU.max)
                    rc = wk.tile([128, 8], F32, tag="rc")
                    nc.vector.reciprocal(rc[:], cntm[:])
                    ea_loop = wk.tile([128, 8], F32, tag="ea_loop")
                    nc.vector.tensor_tensor(ea_loop[:], cntv[:, :, 0:1]
                                            .rearrange("p a b -> p (a b)"),
                                            rc[:], ALU.mult)

                # ---- batched edge logits ----
                ae = wk.tile([EPP, 64], F32, tag="ae")
                nc.gpsimd.tensor_tensor(
                    ae[:].rearrange("p (a h) -> p a h", a=8),
                    c_webe[:, (li - 1) * 64:li * 64]
                    .rearrange("p (a h) -> p a h", a=8),
                    c_ea[:, p0:p0 + 8][:, :, None].broadcast_to([EPP, 8, H]),
                    ALU.mult)
                lg = wk.tile([EPP, 64], F32, tag="lg")
                nc.vector.scalar_tensor_tensor(
                    lg[:], sp[0:112, ASD_:ASD_ + 64], 1.0, ae[:],
                    ALU.mult, ALU.add)
                lg2 = wk.tile([EPP, 64], F32, tag="lg2")
                nc.vector.scalar_tensor_tensor(
                    lg2[:], lg[:], 0.2, lg[:], ALU.mult, ALU.max)
                p_e = wk.tile([EPP, 64], vdt, tag="p_e")
                nc.scalar.activation(p_e[:], lg2[:], ACTF.Exp)

                p_self = None
                if self_loops:
                    sae = wk.tile([128, 64], F32, tag="sae")
                    nc.gpsimd.tensor_tensor(
                        sae[:].rearrange("p (a h) -> p a h", a=8),
                        c_webn[:, (li - 1) * 64:li * 64]
                        .rearrange("p (a h) -> p a h", a=8),
                        ea_loop[:][:, :, None].broadcast_to([128, 8, H]),
                        ALU.mult)
                    s1 = wk.tile([128, 64], F32, tag="s1")
                    nc.gpsimd.tensor_tensor(
                        s1[:].rearrange("p (a h) -> p a h", a=8),
                        hAv[:, :, 256:264],
                        hAv[:, :, 264:272], ALU.add)
                    s2 = wk.tile([128, 64], F32, tag="s2")
                    nc.gpsimd.tensor_tensor(
                        s2[:], s1[:], sae[:], ALU.add)
                    s3 = wk.tile([128, 64], F32, tag="s3")
                    nc.vector.scalar_tensor_tensor(
                        s3[:], s2[:], 0.2, s2[:], ALU.mult, ALU.max)
                    p_self = wk.tile([128, 64], F32, tag="p_self")
                    nc.scalar.activation(p_self[:], s3[:], ACTF.Exp)

                # ---- denominators ----
                for j in range(8):
                    nc.tensor.matmul(sp[:, DEN_ + j * 8:DEN_ + j * 8 + 8],
                                     dtt_l[j][:], p_e[:, j * 8:j * 8 + 8],
                                     start=True, stop=True)
                dtot = wk.tile([128, 64], F32, tag="dtot")
                if self_loops:
                    nc.vector.tensor_tensor(dtot[:], sp[:, DEN_:DEN_ + 64],
                                            p_self[:], ALU.add)
                else:
                    nc.vector.tensor_scalar(dtot[:], sp[:, DEN_:DEN_ + 64],
                                            1e-16, None, ALU.add)
                rden = wk.tile([128, 64], F32, tag="rden")
                nc.vector.reciprocal(rden[:], dtot[:])
                rden_v = wk.tile([128, 64], vdt, tag="rden_v")
                nc.scalar.copy(rden_v[:], rden[:])
                for j in range(8):
                    nc.tensor.matmul(sp[0:112, RD_ + j * 8:RD_ + j * 8 + 8],
                                     dblk_l[j], rden_v[:, j * 8:j * 8 + 8],
                                     start=True, stop=True)
                pn = wk.tile([EPP, 64], vdt, tag="pn")
                nc.vector.tensor_tensor(pn[:], sp[0:112, RD_:RD_ + 64], p_e[:],
                                        ALU.mult)
                if self_loops:
                    psn = wk.tile([128, 64], vdt, tag="psn")
                    nc.gpsimd.tensor_tensor(psn[:], p_self[:], rden[:], ALU.mult)

                # ---- phase B per duet: gather h, messages, scatter, relu ----
                hT_new = []
                for d in range(4):
                    g2 = ps.tile([EPP, 512], F32, tag="g2", bufs=2)
                    for jj in range(2):
                        j = 2 * d + jj
                        nc.tensor.matmul(g2[:, jj * 256:jj * 256 + 256],
                                         sblk_l[j], hAv[:, j, 0:256],
                                         start=True, stop=True)
                    msg2 = wk.tile([EPP, 512], vdt, tag="msg2")
                    nc.vector.tensor_tensor(
                        msg2[:].rearrange("p (a h c) -> p a h c", a=2, h=H),
                        g2[:, :].rearrange("p (a h c) -> p a h c", a=2, h=H),
                        pn[:, d * 16:(d + 1) * 16]
                        .rearrange("p (a h) -> p a h", a=2)[:, :, :, None]
                        .broadcast_to([EPP, 2, H, HID]), ALU.mult)
                    if self_loops:
                        msgs2 = wk.tile([128, 512], vdt, tag="msgs2")
                        for jj in range(2):
                            j = 2 * d + jj
                            nc.gpsimd.tensor_tensor(
                                msgs2[:, jj * 256:(jj + 1) * 256]
                                .rearrange("p (h c) -> p h c", h=H),
                                hAv[:, j, 0:256]
                                .rearrange("p (h c) -> p h c", h=H),
                                psn[:, j * 8:(j + 1) * 8][:, :, None]
                                .broadcast_to([128, H, HID]), ALU.mult)
                    o2 = ps.tile([128, 512], F32, tag="out2", bufs=2)
                    for jj in range(2):
                        j = 2 * d + jj
                        for c in range(2):
                            cs = slice(jj * 256 + c * 128, jj * 256 + c * 128 + 128)
                            nc.tensor.matmul(
                                o2[:, cs], msg2[:, jj * 256 + c * 128:
                                                jj * 256 + (c + 1) * 128],
                                dtt_l[j][:], start=True, stop=not self_loops)
                            if self_loops:
                                nc.tensor.matmul(
                                    o2[:, cs], msgs2[:, jj * 256 + c * 128:
                                                     jj * 256 + (c + 1) * 128],
                                    c_id[:, :], start=False, stop=True)
                    hT2 = wk.tile([128, 512], vdt, tag=f"hT{li}_{d}")
                    nc.scalar.activation(hT2[:], o2[:], ACTF.Relu)
                    hT_new.append(hT2)
                    for jj in range(2):
                        j = 2 * d + jj
                        prevT[j] = [hT2[:, jj * 256:jj * 256 + 128],
                                    hT2[:, jj * 256 + 128:jj * 256 + 256]]
                hT_l3 = hT_new

            # ---- pooling + MLP ----
            gev_l = []
            for d in range(4):
                ge = wk.tile([128, 8], F32, tag=f"ge{d}")
                nc.vector.tensor_reduce(
                    ge[:], hT_l3[d][:, :].rearrange(
                        "p (a b g n) -> p a b g n", a=2, b=2, g=2),
                    mybir.AxisListType.X, ALU.add)
                gev = wk.tile([128, 8], vdt, tag=f"gev{d}")
                nc.scalar.copy(gev[:], ge[:])
                gev_l.append(gev)
            for j in range(8):
                d, jj = j // 2, j % 2
                for c in range(2):
                    agent = hT_l3[d][:, jj * 256 + c * 128:
                                     jj * 256 + (c + 1) * 128] \
                        .rearrange("p (g n) -> p g n", g=2)[:, :, 0:8]
                    nc.tensor.matmul(sp[:, Z_ + j * 16:Z_ + (j + 1) * 16],
                                     c_fc1a[:, bass.ts(c, 128)], agent,
                                     start=(c == 0), stop=(c == 1))
                for c in range(2):
                    nc.tensor.matmul(sp[:, ZG_ + j * 2:ZG_ + (j + 1) * 2],
                                     c_fc1g[:, bass.ts(c, 128)],
                                     gev_l[d][:, jj * 4 + c * 2:jj * 4 + c * 2 + 2],
                                     start=(c == 0), stop=(c == 1))
            zgb = wk.tile([128, 16], F32, tag="zgb")
            nc.vector.scalar_tensor_tensor(
                zgb[:], sp[:, ZG_:ZG_ + 16], 1.0,
                c_fc1b[:, 0:1].broadcast_to([128, 16]), ALU.mult, ALU.add)
            zt = wk.tile([128, 128], F32, tag="zt")
            nc.vector.scalar_tensor_tensor(
                zt[:].rearrange("p (a b) -> p a b", a=16),
                sp[:, Z_:Z_ + 128].rearrange("p (a b) -> p a b", a=16), 1.0,
                zgb[:][:, :, None].broadcast_to([128, 16, 8]),
                ALU.mult, ALU.add)
            zbat = wk.tile([128, 128], vdt, tag="zbat")
            nc.scalar.activation(zbat[:], zt[:], ACTF.Relu)
            nc.tensor.matmul(sp[0:OUT, 0:128], c_fc2w[:, :], zbat[:],
                             start=True, stop=True)
            nc.vector.tensor_scalar(out_acc[:, oct_i * 128:(oct_i + 1) * 128],
                                    sp[0:OUT, 0:128], c_fc2b[:, 0:1], None,
                                    ALU.add)

        nc.sync.dma_start(out_d[:, :], out_acc[:])

    nc.compile()
    return nc


# ---------------- host-side packing ----------------

def _np_vdt(vdt):
    import ml_dtypes
    return {mybir.dt.bfloat16: ml_dtypes.bfloat16,
            mybir.dt.float32: np.float32}[vdt]


def host_prep(inputs, npairs=GPC // 2, vdt=mybir.dt.bfloat16):
    nv = _np_vdt(vdt)
    x = np.asarray(inputs["x"], np.float32)
    ei = np.asarray(inputs["edge_index"])
    eattr = np.asarray(inputs["edge_attr"], np.float32)
    for l in (1, 2, 3):
        assert not np.any(np.asarray(inputs[f"b{l}"])), "GAT bias must be 0"

    def pack_w(l):
        W = np.asarray(inputs[f"W{l}"], np.float32)
        a_s = np.asarray(inputs[f"as{l}"], np.float32)
        a_d = np.asarray(inputs[f"ad{l}"], np.float32)
        Ps = np.einsum("fkc,kc->fk", W.reshape(W.shape[0], H, HID), a_s)
        Pd = np.einsum("fkc,kc->fk", W.reshape(W.shape[0], H, HID), a_d)
        return np.concatenate([W, Ps, Pd], axis=1).astype(nv)

    def w_e(l):
        We = np.asarray(inputs[f"We{l}"], np.float32).reshape(H, HID)
        a_e = np.asarray(inputs[f"ae{l}"], np.float32)
        return (We * a_e).sum(-1)

    waug = {l: pack_w(l) for l in (1, 2, 3)}
    for l in (2, 3):
        waug[l] = np.concatenate([waug[l][:128], waug[l][128:]], axis=1)
    wev = np.concatenate([np.tile(w_e(l), 8) for l in (1, 2, 3)])   # [192]
    webe = np.broadcast_to(wev, (EPP, 192)).astype(np.float32).copy()
    webn = np.broadcast_to(wev, (128, 192)).astype(np.float32).copy()
    fc1_w = np.asarray(inputs["fc1_w"], np.float32)
    fc1a = np.concatenate([fc1_w[:128], fc1_w[128:HC]], axis=1).astype(nv)
    fc1g = np.concatenate([fc1_w[HC:HC + 128] / P,
                           fc1_w[HC + 128:] / P], axis=1).astype(nv)
    fc1b = np.asarray(inputs["fc1_b"], np.float32).reshape(128, 1)
    fc2w = np.asarray(inputs["fc2_w"], np.float32).astype(nv)
    fc2b = np.asarray(inputs["fc2_b"], np.float32).reshape(OUT, 1)
    identm = np.eye(128, dtype=np.float32).astype(nv)
    iota = np.broadcast_to(np.arange(128, dtype=np.float32), (EPP, 128)).copy()
    iotac = np.arange(128, dtype=np.float32).reshape(128, 1)

    maps = []
    npc = GPC * P
    epc = GPC * OBS
    for m in range(NCORES):
        nsl = slice(m * npc, (m + 1) * npc)
        esl = slice(m * epc, (m + 1) * epc)
        xt = np.ascontiguousarray(x[nsl].T).astype(nv)
        src = np.asarray(ei[0][esl], np.int64) - m * npc
        dst = np.asarray(ei[1][esl], np.int64) - m * npc
        pairs = np.arange(GPC // 2).repeat(EPP)
        src_l = (src.reshape(-1) - pairs * 128).astype(np.float32)
        dst_l = (dst.reshape(-1) - pairs * 128).astype(np.float32)
        esrcb = np.ascontiguousarray(src_l.reshape(-1, EPP))
        edstb = np.ascontiguousarray(dst_l.reshape(-1, EPP))
        edst = np.ascontiguousarray(dst_l.reshape(-1, EPP).T)
        eat = np.ascontiguousarray(eattr[esl].reshape(-1, EPP).T).astype(np.float32)
        eap_arr = np.empty((EPP, 2 * npairs), np.float32)
        eap_arr[:, 0::2] = eat[:, :npairs]
        eap_arr[:, 1::2] = 1.0
        maps.append({
            "xt": xt[:, :npairs * 128],
            "esrcb": esrcb[:npairs], "edstb": edstb[:npairs],
            "edst": edst[:, :npairs],
            "eattr": eat[:, :npairs], "eap": eap_arr.astype(nv),
            "waug1": waug[1], "waug2": waug[2], "waug3": waug[3],
            "webe": webe, "webn": webn,
            "fc1a": fc1a, "fc1g": fc1g, "fc1b": fc1b,
            "fc2w": fc2w, "fc2b": fc2b,
            "ident": identm, "iota": iota, "iotac": iotac,
        })
    return maps


def unpack_out(res_list, npairs=GPC // 2):
    outs = []
    for m in range(NCORES):
        o = res_list[m]["out"]
        o = o.reshape(OUT, npairs, 2, A).transpose(1, 2, 3, 0)
        outs.append(o.reshape(npairs * 2, A, OUT))
    return np.concatenate(outs, axis=0).astype(np.float32)


# ---------------- entry point ----------------

LAST_EXEC_NS = None
_NC_CACHE = {}


def kernel(**inputs) -> np.ndarray:
    """Full-input GAT forward on 8 NeuronCores; returns [4096, 8, 2] f32."""
    global LAST_EXEC_NS
    import os
    vdt = mybir.dt.bfloat16
    npairs = GPC // 2
    key = (npairs, vdt)
    if key not in _NC_CACHE:
        _NC_CACHE[key] = build(npairs, vdt=vdt, num_devices=NCORES)
    nc = _NC_CACHE[key]
    maps = host_prep(inputs, npairs=npairs, vdt=vdt)
    trace = os.environ.get("BASS_GAT_TRACE") == "1"
    res = None
    for attempt in range(3):
        try:
            res = run_bass_kernel_spmd(
                nc, maps, core_ids=list(range(NCORES)),
                trace=trace and attempt == 0,
                trace_cores=[0] if trace and attempt == 0 else None)
            break
        except Exception:
            if attempt == 2:
                raise
            import time
            time.sleep(10)
    LAST_EXEC_NS = res.exec_time_ns
    return unpack_out([r for r in res.results], npairs=npairs)


# revision 5
# speedup vs baseline: 1.5180x; 1.0191x over previous
"""GAT model Bass/Tile kernel for TRN2 (self-contained, octet-batched).

Per core: 512 graphs as 256 pairs (128 nodes / 112 edges). Pairs are
processed in octets (8 pairs): per-edge/per-node attention scalars are
batched into [*, 64] ops across the octet; fat value ops run at duet
(2-pair) granularity; engines are balanced DVE/ACT/GPSIMD/PE.
"""
import numpy as np
from contextlib import ExitStack

import concourse.bass as bass
import concourse.tile as tile
from concourse import bacc, mybir
from concourse.bass_utils import run_bass_kernel_spmd

F32 = mybir.dt.float32
I32 = mybir.dt.int32

B, A, OBS = 4096, 8, 56
P = 64
H, HID, HC = 8, 32, 256
IN, OUT = 16, 2
NCORES = 8
GPC = B // NCORES
EPP = 2 * OBS
ALU = mybir.AluOpType
ACTF = mybir.ActivationFunctionType

# small_ps column regions (f32); Z/ZG/oc reuse the same tile post-L3
ASD_, DEN_, RD_, CNT_, Z_, ZG_ = 0, 64, 128, 192, 0, 128


def build(npairs: int, vdt=mybir.dt.bfloat16, num_devices: int = NCORES):
    assert npairs % 8 == 0
    nc = bacc.Bacc("TRN2", target_bir_lowering=False, debug=False,
                   num_devices=num_devices)
    NP = npairs

    def din(name, shape, dt):
        return nc.dram_tensor(name, shape, dt, kind="ExternalInput").ap()

    xt = din("xt", [IN, NP * 128], vdt)
    esrcb = din("esrcb", [NP, EPP], F32)
    edstb = din("edstb", [NP, EPP], F32)
    edst = din("edst", [EPP, NP], F32)
    eattr = din("eattr", [EPP, NP], F32)
    eap = din("eap", [EPP, 2 * NP], vdt)
    waug1 = din("waug1", [IN, 272], vdt)
    waug2 = din("waug2", [128, 544], vdt)
    waug3 = din("waug3", [128, 544], vdt)
    webe = din("webe", [EPP, 3 * 64], F32)    # w_e tiled 8x per layer
    webn = din("webn", [128, 3 * 64], F32)
    fc1a = din("fc1a", [128, HC], vdt)
    fc1g = din("fc1g", [128, HC], vdt)
    fc1b = din("fc1b", [128, 1], F32)
    fc2w = din("fc2w", [128, OUT], vdt)
    fc2b = din("fc2b", [OUT, 1], F32)
    ident = din("ident", [128, 128], vdt)
    iota = din("iota", [EPP, 128], F32)
    iotac = din("iotac", [128, 1], F32)

    out_d = nc.dram_tensor("out", [OUT, NP * 16], F32, kind="ExternalOutput").ap()

    with tile.TileContext(nc) as tc, ExitStack() as ctx:
        cpool = ctx.enter_context(tc.tile_pool(name="const", bufs=1))
        wk = ctx.enter_context(tc.tile_pool(name="work", bufs=4))
        eb = ctx.enter_context(tc.tile_pool(name="edges", bufs=24))
        ps = ctx.enter_context(tc.tile_pool(name="psum", bufs=1, space="PSUM"))

        def cload(ap, tag):
            t = cpool.tile(list(ap.shape), ap.dtype, tag=tag)
            nc.sync.dma_start(t[:], ap[:, :])
            return t

        c_w1, c_w2, c_w3 = cload(waug1, "w1"), cload(waug2, "w2"), cload(waug3, "w3")
        c_webe, c_webn = cload(webe, "webe"), cload(webn, "webn")
        c_fc1a, c_fc1g = cload(fc1a, "fc1a"), cload(fc1g, "fc1g")
        c_fc1b, c_fc2w, c_fc2b = cload(fc1b, "fc1b"), cload(fc2w, "fc2w"), cload(fc2b, "fc2b")
        c_id, c_iota = cload(ident, "ident"), cload(iota, "iota")
        c_iotac = cload(iotac, "iotac")
        c_edst = cload(edst, "edst")
        c_ea, c_eap = cload(eattr, "eattr"), cload(eap, "eap")

        out_acc = cpool.tile([OUT, NP * 16], F32, tag="out_acc")

        wchunks = {1: [c_w1[:, :]],
                   2: [c_w2[:, 0:272], c_w2[:, 272:544]],
                   3: [c_w3[:, 0:272], c_w3[:, 272:544]]}

        for oct_i in range(NP // 8):
            p0 = oct_i * 8

            # ---- phase A: edge structure + x loads ----
            srcb = eb.tile([128, 8 * EPP], F32, tag="srcb", bufs=2)
            nc.sync.dma_start(srcb[:], esrcb[p0:p0 + 8, :]
                              .rearrange("a b -> (a b)")[None, :]
                              .broadcast_to([128, 8 * EPP]))
            dstb = eb.tile([128, 8 * EPP], F32, tag="dstb", bufs=2)
            nc.sync.dma_start(dstb[:], edstb[p0:p0 + 8, :]
                              .rearrange("a b -> (a b)")[None, :]
                              .broadcast_to([128, 8 * EPP]))
            sblk_o = eb.tile([128, 8 * EPP], vdt, tag="sblk_o", bufs=2)
            nc.vector.tensor_scalar(sblk_o[:], srcb[:], c_iotac[:, 0:1],
                                    None, ALU.is_equal)
            dblk_o = eb.tile([128, 8 * EPP], vdt, tag="dblk_o", bufs=2)
            nc.vector.tensor_scalar(dblk_o[:], dstb[:], c_iotac[:, 0:1],
                                    None, ALU.is_equal)
            sblk_l = [sblk_o[:, j * EPP:(j + 1) * EPP] for j in range(8)]
            dblk_l = [dblk_o[:, j * EPP:(j + 1) * EPP] for j in range(8)]
            dtt_l, x0_l = [], []
            for j in range(8):
                pp = p0 + j
                dtt = eb.tile([EPP, 128], vdt, tag="dtt")
                nc.vector.tensor_scalar(dtt[:], c_iota[:], c_edst[:, pp:pp + 1],
                                        None, ALU.is_equal)
                x0 = eb.tile([IN, 128], vdt, tag="x0")
                nc.sync.dma_start(x0[:], xt[:, pp * 128:(pp + 1) * 128])
                dtt_l.append(dtt); x0_l.append(x0)

            prevT = [[x0_l[j][:, :]] for j in range(8)]
            ea_loop = None
            hT_l3 = None

            for li in (1, 2, 3):
                wch = wchunks[li]
                self_loops = li > 1
                sp = ps.tile([128, 208], F32, tag="small", bufs=3)
                hAso = wk.tile([128, 8 * 272], vdt, tag="hAso", bufs=3)
                hAv = hAso[:, :].rearrange("p (pr x) -> p pr x", pr=8)

                # ---- MM1 per pair + hAs copy + small gathers ----
                for j in range(8):
                    h2 = ps.tile([128, 272], F32, tag="h2", bufs=2)
                    for kc, (sta, wc) in enumerate(zip(prevT[j], wch)):
                        nc.tensor.matmul(h2[:], sta, wc, start=(kc == 0),
                                         stop=(kc == len(wch) - 1))
                    if j % 2 == 0:
                        nc.vector.tensor_copy(
                            hAso[:, j * 272:(j + 1) * 272], h2[:])
                    else:
                        nc.scalar.copy(
                            hAso[:, j * 272:(j + 1) * 272], h2[:])
                    nc.tensor.matmul(sp[0:112, ASD_ + j * 8:ASD_ + j * 8 + 8],
                                     sblk_l[j], hAv[:, j, 256:264],
                                     start=True, stop=False)
                    nc.tensor.matmul(sp[0:112, ASD_ + j * 8:ASD_ + j * 8 + 8],
                                     dblk_l[j], hAv[:, j, 264:272],
                                     start=False, stop=True)
                    if li == 1:
                        nc.tensor.matmul(
                            sp[:, CNT_ + j * 2:CNT_ + j * 2 + 2], dtt_l[j][:],
                            c_eap[:, 2 * (p0 + j):2 * (p0 + j) + 2],
                            start=True, stop=True)

                if li == 1:
                    cntv = sp[:, CNT_:CNT_ + 16].rearrange(
                        "p (pr two) -> p pr two", two=2)
                    cntm = wk.tile([128, 8], F32, tag="cntm")
                    nc.vector.tensor_scalar(cntm[:], cntv[:, :, 1:2], 1.0,
                                            None, ALU.max)
                    rc = wk.tile([128, 8], F32, tag="rc")
                    nc.vector.reciprocal(rc[:], cntm[:])
                    ea_loop = wk.tile([128, 8], F32, tag="ea_loop")
                    nc.vector.tensor_tensor(ea_loop[:], cntv[:, :, 0:1]
                                            .rearrange("p a b -> p (a b)"),
                                            rc[:], ALU.mult)

                # ---- batched edge logits ----
                ae = wk.tile([EPP, 64], F32, tag="ae")
                nc.gpsimd.tensor_tensor(
                    ae[:].rearrange("p (a h) -> p a h", a=8),
                    c_webe[:, (li - 1) * 64:li * 64]
                    .rearrange("p (a h) -> p a h", a=8),
                    c_ea[:, p0:p0 + 8][:, :, None].broadcast_to([EPP, 8, H]),
                    ALU.mult)
                lg = wk.tile([EPP, 64], F32, tag="lg")
                nc.vector.scalar_tensor_tensor(
                    lg[:], sp[0:112, ASD_:ASD_ + 64], 1.0, ae[:],
                    ALU.mult, ALU.add)
                lg2 = wk.tile([EPP, 64], F32, tag="lg2")
                nc.vector.scalar_tensor_tensor(
                    lg2[:], lg[:], 0.2, lg[:], ALU.mult, ALU.max)
                p_e = wk.tile([EPP, 64], vdt, tag="p_e")
                nc.scalar.activation(p_e[:], lg2[:], ACTF.Exp)

                p_self = None
                if self_loops:
                    sae = wk.tile([128, 64], F32, tag="sae")
                    nc.gpsimd.tensor_tensor(
                        sae[:].rearrange("p (a h) -> p a h", a=8),
                        c_webn[:, (li - 1) * 64:li * 64]
                        .rearrange("p (a h) -> p a h", a=8),
                        ea_loop[:][:, :, None].broadcast_to([128, 8, H]),
                        ALU.mult)
                    s1 = wk.tile([128, 64], F32, tag="s1")
                    nc.gpsimd.tensor_tensor(
                        s1[:].rearrange("p (a h) -> p a h", a=8),
                        hAv[:, :, 256:264],
                        hAv[:, :, 264:272], ALU.add)
                    s2 = wk.tile([128, 64], F32, tag="s2")
                    nc.gpsimd.tensor_tensor(
                        s2[:], s1[:], sae[:], ALU.add)
                    s3 = wk.tile([128, 64], F32, tag="s3")
                    nc.vector.scalar_tensor_tensor(
                        s3[:], s2[:], 0.2, s2[:], ALU.mult, ALU.max)
                    p_self = wk.tile([128, 64], F32, tag="p_self")
                    nc.scalar.activation(p_self[:], s3[:], ACTF.Exp)

                # ---- denominators ----
                for j in range(8):
                    nc.tensor.matmul(sp[:, DEN_ + j * 8:DEN_ + j * 8 + 8],
                                     dtt_l[j][:], p_e[:, j * 8:j * 8 + 8],
                                     start=True, stop=True)
                dtot = wk.tile([128, 64], F32, tag="dtot")
                if self_loops:
                    nc.vector.tensor_tensor(dtot[:], sp[:, DEN_:DEN_ + 64],
                                            p_self[:], ALU.add)
                else:
                    nc.vector.tensor_scalar(dtot[:], sp[:, DEN_:DEN_ + 64],
                                            1e-16, None, ALU.add)
                rden = wk.tile([128, 64], F32, tag="rden")
                nc.vector.reciprocal(rden[:], dtot[:])
                rden_v = wk.tile([128, 64], vdt, tag="rden_v")
                nc.scalar.copy(rden_v[:], rden[:])
                for j in range(8):
                    nc.tensor.matmul(sp[0:112, RD_ + j * 8:RD_ + j * 8 + 8],
                                     dblk_l[j], rden_v[:, j * 8:j * 8 + 8],
                                     start=True, stop=True)
                pn = wk.tile([EPP, 64], vdt, tag="pn")
                nc.vector.tensor_tensor(pn[:], sp[0:112, RD_:RD_ + 64], p_e[:],
                                        ALU.mult)
                if self_loops:
                    psn = wk.tile([128, 64], vdt, tag="psn")
                    nc.gpsimd.tensor_tensor(psn[:], p_self[:], rden[:], ALU.mult)

                # ---- phase B per duet: gather h, messages, scatter, relu ----
                hT_new = []
                for d in range(4):
                    g2 = ps.tile([EPP, 512], F32, tag="g2", bufs=2)
                    for jj in range(2):
                        j = 2 * d + jj
                        nc.tensor.matmul(g2[:, jj * 256:jj * 256 + 256],
                                         sblk_l[j], hAv[:, j, 0:256],
                                         start=True, stop=True)
                    msg2 = wk.tile([EPP, 512], vdt, tag="msg2")
                    nc.vector.tensor_tensor(
                        msg2[:].rearrange("p (a h c) -> p a h c", a=2, h=H),
                        g2[:, :].rearrange("p (a h c) -> p a h c", a=2, h=H),
                        pn[:, d * 16:(d + 1) * 16]
                        .rearrange("p (a h) -> p a h", a=2)[:, :, :, None]
                        .broadcast_to([EPP, 2, H, HID]), ALU.mult)
                    if self_loops:
                        msgs2 = wk.tile([128, 512], vdt, tag="msgs2")
                        for jj in range(2):
                            j = 2 * d + jj
                            nc.gpsimd.tensor_tensor(
                                msgs2[:, jj * 256:(jj + 1) * 256]
                                .rearrange("p (h c) -> p h c", h=H),
                                hAv[:, j, 0:256]
                                .rearrange("p (h c) -> p h c", h=H),
                                psn[:, j * 8:(j + 1) * 8][:, :, None]
                                .broadcast_to([128, H, HID]), ALU.mult)
                    o2 = ps.tile([128, 512], F32, tag="out2", bufs=1)
                    for jj in range(2):
                        j = 2 * d + jj
                        for c in range(2):
                            cs = slice(jj * 256 + c * 128, jj * 256 + c * 128 + 128)
                            nc.tensor.matmul(
                                o2[:, cs], msg2[:, jj * 256 + c * 128:
                                                jj * 256 + (c + 1) * 128],
                                dtt_l[j][:], start=True, stop=not self_loops)
                            if self_loops:
                                nc.tensor.matmul(
                                    o2[:, cs], msgs2[:, jj * 256 + c * 128:
                                                     jj * 256 + (c + 1) * 128],
                                    c_id[:, :], start=False, stop=True)
                    hT2 = wk.tile([128, 512], vdt, tag=f"hT{li}_{d}")
                    nc.scalar.activation(hT2[:], o2[:], ACTF.Relu)
                    hT_new.append(hT2)
                    for jj in range(2):
                        j = 2 * d + jj
                        prevT[j] = [hT2[:, jj * 256:jj * 256 + 128],
                                    hT2[:, jj * 256 + 128:jj * 256 + 256]]
                hT_l3 = hT_new

            # ---- pooling + MLP (octet-batched) ----
            zmlp = ps.tile([128, 144], F32, tag="g2", bufs=2)
            gev_o = wk.tile([128, 32], F32, tag="gev_o")   # (c, pair, g)
            gvv = gev_o[:, :].rearrange("p (c pr g) -> p pr c g", c=2, g=2)
            for d in range(4):
                nc.vector.tensor_reduce(
                    gvv[:, 2 * d:2 * d + 2, :, :],
                    hT_l3[d][:, :].rearrange(
                        "p (a b g n) -> p a b g n", a=2, b=2, g=2),
                    mybir.AxisListType.X, ALU.add)
            gev_v = wk.tile([128, 32], vdt, tag="gev_v")
            nc.scalar.copy(gev_v[:], gev_o[:])
            for d in range(4):
                for c in range(2):
                    agent = hT_l3[d][:, :].rearrange(
                        "p (a b g n) -> p a b g n", a=2, b=2, g=2)[:, :, c, :, 0:8]
                    nc.tensor.matmul(zmlp[:, Z_ + d * 32:Z_ + (d + 1) * 32],
                                     c_fc1a[:, bass.ts(c, 128)], agent,
                                     start=(c == 0), stop=(c == 1))
            for c in range(2):
                nc.tensor.matmul(zmlp[:, ZG_:ZG_ + 16],
                                 c_fc1g[:, bass.ts(c, 128)],
                                 gev_v[:, c * 16:(c + 1) * 16],
                                 start=(c == 0), stop=(c == 1))
            zgb = wk.tile([128, 16], F32, tag="zgb")
            nc.vector.scalar_tensor_tensor(
                zgb[:], zmlp[:, ZG_:ZG_ + 16], 1.0,
                c_fc1b[:, 0:1].broadcast_to([128, 16]), ALU.mult, ALU.add)
            zt = wk.tile([128, 128], F32, tag="zt")
            nc.vector.scalar_tensor_tensor(
                zt[:].rearrange("p (a b) -> p a b", a=16),
                zmlp[:, Z_:Z_ + 128].rearrange("p (a b) -> p a b", a=16), 1.0,
                zgb[:][:, :, None].broadcast_to([128, 16, 8]),
                ALU.mult, ALU.add)
            zbat = wk.tile([128, 128], vdt, tag="zbat")
            nc.scalar.activation(zbat[:], zt[:], ACTF.Relu)
            nc.tensor.matmul(zmlp[0:OUT, 0:128], c_fc2w[:, :], zbat[:],
                             start=True, stop=True)
            nc.vector.tensor_scalar(out_acc[:, oct_i * 128:(oct_i + 1) * 128],
                                    zmlp[0:OUT, 0:128], c_fc2b[:, 0:1], None,
                                    ALU.add)

        nc.sync.dma_start(out_d[:, :], out_acc[:])

    nc.compile()
    return nc


# ---------------- host-side packing ----------------

def _np_vdt(vdt):
    import ml_dtypes
    return {mybir.dt.bfloat16: ml_dtypes.bfloat16,
            mybir.dt.float32: np.float32}[vdt]


def host_prep(inputs, npairs=GPC // 2, vdt=mybir.dt.bfloat16):
    nv = _np_vdt(vdt)
    x = np.asarray(inputs["x"], np.float32)
    ei = np.asarray(inputs["edge_index"])
    eattr = np.asarray(inputs["edge_attr"], np.float32)
    for l in (1, 2, 3):
        assert not np.any(np.asarray(inputs[f"b{l}"])), "GAT bias must be 0"

    def pack_w(l):
        W = np.asarray(inputs[f"W{l}"], np.float32)
        a_s = np.asarray(inputs[f"as{l}"], np.float32)
        a_d = np.asarray(inputs[f"ad{l}"], np.float32)
        Ps = np.einsum("fkc,kc->fk", W.reshape(W.shape[0], H, HID), a_s)
        Pd = np.einsum("fkc,kc->fk", W.reshape(W.shape[0], H, HID), a_d)
        return np.concatenate([W, Ps, Pd], axis=1).astype(nv)

    def w_e(l):
        We = np.asarray(inputs[f"We{l}"], np.float32).reshape(H, HID)
        a_e = np.asarray(inputs[f"ae{l}"], np.float32)
        return (We * a_e).sum(-1)

    waug = {l: pack_w(l) for l in (1, 2, 3)}
    for l in (2, 3):
        waug[l] = np.concatenate([waug[l][:128], waug[l][128:]], axis=1)
    wev = np.concatenate([np.tile(w_e(l), 8) for l in (1, 2, 3)])   # [192]
    webe = np.broadcast_to(wev, (EPP, 192)).astype(np.float32).copy()
    webn = np.broadcast_to(wev, (128, 192)).astype(np.float32).copy()
    fc1_w = np.asarray(inputs["fc1_w"], np.float32)
    fc1a = np.concatenate([fc1_w[:128], fc1_w[128:HC]], axis=1).astype(nv)
    fc1g = np.concatenate([fc1_w[HC:HC + 128] / P,
                           fc1_w[HC + 128:] / P], axis=1).astype(nv)
    fc1b = np.asarray(inputs["fc1_b"], np.float32).reshape(128, 1)
    fc2w = np.asarray(inputs["fc2_w"], np.float32).astype(nv)
    fc2b = np.asarray(inputs["fc2_b"], np.float32).reshape(OUT, 1)
    identm = np.eye(128, dtype=np.float32).astype(nv)
    iota = np.broadcast_to(np.arange(128, dtype=np.float32), (EPP, 128)).copy()
    iotac = np.arange(128, dtype=np.float32).reshape(128, 1)

    maps = []
    npc = GPC * P
    epc = GPC * OBS
    for m in range(NCORES):
        nsl = slice(m * npc, (m + 1) * npc)
        esl = slice(m * epc, (m + 1) * epc)
        xt = np.ascontiguousarray(x[nsl].T).astype(nv)
        src = np.asarray(ei[0][esl], np.int64) - m * npc
        dst = np.asarray(ei[1][esl], np.int64) - m * npc
        pairs = np.arange(GPC // 2).repeat(EPP)
        src_l = (src.reshape(-1) - pairs * 128).astype(np.float32)
        dst_l = (dst.reshape(-1) - pairs * 128).astype(np.float32)
        esrcb = np.ascontiguousarray(src_l.reshape(-1, EPP))
        edstb = np.ascontiguousarray(dst_l.reshape(-1, EPP))
        edst = np.ascontiguousarray(dst_l.reshape(-1, EPP).T)
        eat = np.ascontiguousarray(eattr[esl].reshape(-1, EPP).T).astype(np.float32)
        eap_arr = np.empty((EPP, 2 * npairs), np.float32)
        eap_arr[:, 0::2] = eat[:, :npairs]
        eap_arr[:, 1::2] = 1.0
        maps.append({
            "xt": xt[:, :npairs * 128],
            "esrcb": esrcb[:npairs], "edstb": edstb[:npairs],
            "edst": edst[:, :npairs],
            "eattr": eat[:, :npairs], "eap": eap_arr.astype(nv),
            "waug1": waug[1], "waug2": waug[2], "waug3": waug[3],
            "webe": webe, "webn": webn,
            "fc1a": fc1a, "fc1g": fc1g, "fc1b": fc1b,
            "fc2w": fc2w, "fc2b": fc2b,
            "ident": identm, "iota": iota, "iotac": iotac,
        })
    return maps


def unpack_out(res_list, npairs=GPC // 2):
    outs = []
    for m in range(NCORES):
        o = res_list[m]["out"]
        o = o.reshape(OUT, npairs, 2, A).transpose(1, 2, 3, 0)
        outs.append(o.reshape(npairs * 2, A, OUT))
    return np.concatenate(outs, axis=0).astype(np.float32)


# ---------------- entry point ----------------

LAST_EXEC_NS = None
_NC_CACHE = {}


def kernel(**inputs) -> np.ndarray:
    """Full-input GAT forward on 8 NeuronCores; returns [4096, 8, 2] f32."""
    global LAST_EXEC_NS
    import os
    vdt = mybir.dt.bfloat16
    npairs = GPC // 2
    key = (npairs, vdt)
    if key not in _NC_CACHE:
        _NC_CACHE[key] = build(npairs, vdt=vdt, num_devices=NCORES)
    nc = _NC_CACHE[key]
    maps = host_prep(inputs, npairs=npairs, vdt=vdt)
    trace = os.environ.get("BASS_GAT_TRACE") == "1"
    res = None
    for attempt in range(3):
        try:
            res = run_bass_kernel_spmd(
                nc, maps, core_ids=list(range(NCORES)),
                trace=trace and attempt == 0,
                trace_cores=[0] if trace and attempt == 0 else None)
            break
        except Exception:
            if attempt == 2:
                raise
            import time
            time.sleep(10)
    LAST_EXEC_NS = res.exec_time_ns
    return unpack_out([r for r in res.results], npairs=npairs)


# revision 6
# speedup vs baseline: 1.5611x; 1.0284x over previous
"""GAT model Bass/Tile kernel for TRN2 (self-contained, octet-batched).

Per core: 512 graphs as 256 pairs (128 nodes / 112 edges). Pairs are
processed in octets (8 pairs): per-edge/per-node attention scalars are
batched into [*, 64] ops across the octet; fat value ops run at duet
(2-pair) granularity; engines are balanced DVE/ACT/GPSIMD/PE.
"""
import numpy as np
from contextlib import ExitStack

import concourse.bass as bass
import concourse.tile as tile
from concourse import bacc, mybir
from concourse.bass_utils import run_bass_kernel_spmd

F32 = mybir.dt.float32
I32 = mybir.dt.int32

B, A, OBS = 4096, 8, 56
P = 64
H, HID, HC = 8, 32, 256
IN, OUT = 16, 2
NCORES = 8
GPC = B // NCORES
EPP = 2 * OBS
ALU = mybir.AluOpType
ACTF = mybir.ActivationFunctionType

# small_ps column regions (f32); Z/ZG/oc reuse the same tile post-L3
ASD_, DEN_, RD_, CNT_, Z_, ZG_ = 0, 64, 128, 192, 0, 128


def build(npairs: int, vdt=mybir.dt.bfloat16, num_devices: int = NCORES):
    assert npairs % 8 == 0
    nc = bacc.Bacc("TRN2", target_bir_lowering=False, debug=False,
                   num_devices=num_devices)
    NP = npairs

    def din(name, shape, dt):
        return nc.dram_tensor(name, shape, dt, kind="ExternalInput").ap()

    xt = din("xt", [IN, NP * 128], vdt)
    esrcb = din("esrcb", [NP, EPP], vdt)
    edstb = din("edstb", [NP, EPP], vdt)
    edst = din("edst", [EPP, NP], F32)
    eattr = din("eattr", [EPP, NP], F32)
    eap = din("eap", [EPP, 2 * NP], vdt)
    waug1 = din("waug1", [IN, 272], vdt)
    waug2 = din("waug2", [128, 544], vdt)
    waug3 = din("waug3", [128, 544], vdt)
    webe = din("webe", [EPP, 3 * 64], F32)    # w_e tiled 8x per layer
    webn = din("webn", [128, 3 * 64], F32)
    fc1a = din("fc1a", [128, HC], vdt)
    fc1g = din("fc1g", [128, HC], vdt)
    fc1b = din("fc1b", [128, 1], F32)
    fc2w = din("fc2w", [128, OUT], vdt)
    fc2b = din("fc2b", [OUT, 1], F32)
    ident = din("ident", [128, 128], vdt)
    iota = din("iota", [EPP, 128], vdt)
    iotac = din("iotac", [128, 1], F32)

    out_d = nc.dram_tensor("out", [OUT, NP * 16], F32, kind="ExternalOutput").ap()

    with tile.TileContext(nc) as tc, ExitStack() as ctx:
        cpool = ctx.enter_context(tc.tile_pool(name="const", bufs=1))
        wk = ctx.enter_context(tc.tile_pool(name="work", bufs=4))
        eb = ctx.enter_context(tc.tile_pool(name="edges", bufs=24))
        ps = ctx.enter_context(tc.tile_pool(name="psum", bufs=1, space="PSUM"))

        def cload(ap, tag):
            t = cpool.tile(list(ap.shape), ap.dtype, tag=tag)
            nc.sync.dma_start(t[:], ap[:, :])
            return t

        c_w1, c_w2, c_w3 = cload(waug1, "w1"), cload(waug2, "w2"), cload(waug3, "w3")
        c_webe, c_webn = cload(webe, "webe"), cload(webn, "webn")
        c_fc1a, c_fc1g = cload(fc1a, "fc1a"), cload(fc1g, "fc1g")
        c_fc1b, c_fc2w, c_fc2b = cload(fc1b, "fc1b"), cload(fc2w, "fc2w"), cload(fc2b, "fc2b")
        c_id, c_iota = cload(ident, "ident"), cload(iota, "iota")
        c_iotac = cload(iotac, "iotac")
        c_edst = cload(edst, "edst")
        c_ea, c_eap = cload(eattr, "eattr"), cload(eap, "eap")

        out_acc = cpool.tile([OUT, NP * 16], F32, tag="out_acc")

        wchunks = {1: [c_w1[:, :]],
                   2: [c_w2[:, 0:272], c_w2[:, 272:544]],
                   3: [c_w3[:, 0:272], c_w3[:, 272:544]]}

        for oct_i in range(NP // 8):
            p0 = oct_i * 8

            # ---- phase A: edge structure + x loads ----
            srcb = eb.tile([128, 8 * EPP], vdt, tag="srcb", bufs=2)
            nc.sync.dma_start(srcb[:], esrcb[p0:p0 + 8, :]
                              .rearrange("a b -> (a b)")[None, :]
                              .broadcast_to([128, 8 * EPP]))
            dstb = eb.tile([128, 8 * EPP], vdt, tag="dstb", bufs=2)
            nc.sync.dma_start(dstb[:], edstb[p0:p0 + 8, :]
                              .rearrange("a b -> (a b)")[None, :]
                              .broadcast_to([128, 8 * EPP]))
            sblk_o = eb.tile([128, 8 * EPP], vdt, tag="sblk_o", bufs=2)
            nc.vector.tensor_scalar(sblk_o[:], srcb[:], c_iotac[:, 0:1],
                                    None, ALU.is_equal)
            dblk_o = eb.tile([128, 8 * EPP], vdt, tag="dblk_o", bufs=2)
            nc.vector.tensor_scalar(dblk_o[:], dstb[:], c_iotac[:, 0:1],
                                    None, ALU.is_equal)
            sblk_l = [sblk_o[:, j * EPP:(j + 1) * EPP] for j in range(8)]
            dblk_l = [dblk_o[:, j * EPP:(j + 1) * EPP] for j in range(8)]
            dtt_l, x0_l = [], []
            for j in range(8):
                pp = p0 + j
                dtt = eb.tile([EPP, 128], vdt, tag="dtt")
                nc.vector.tensor_scalar(dtt[:], c_iota[:], c_edst[:, pp:pp + 1],
                                        None, ALU.is_equal)
                x0 = eb.tile([IN, 128], vdt, tag="x0")
                nc.sync.dma_start(x0[:], xt[:, pp * 128:(pp + 1) * 128])
                dtt_l.append(dtt); x0_l.append(x0)

            prevT = [[x0_l[j][:, :]] for j in range(8)]
            ea_loop = None
            hT_l3 = None

            for li in (1, 2, 3):
                wch = wchunks[li]
                self_loops = li > 1
                sp = ps.tile([128, 208], F32, tag="small", bufs=3)
                hAso = wk.tile([128, 8 * 272], vdt, tag="hAso", bufs=3)
                hAv = hAso[:, :].rearrange("p (pr x) -> p pr x", pr=8)

                # ---- MM1 per pair + hAs copy + small gathers ----
                for j in range(8):
                    h2 = ps.tile([128, 272], F32, tag="h2", bufs=2)
                    for kc, (sta, wc) in enumerate(zip(prevT[j], wch)):
                        nc.tensor.matmul(h2[:], sta, wc, start=(kc == 0),
                                         stop=(kc == len(wch) - 1))
                    if j % 4 == 0:
                        nc.vector.tensor_copy(
                            hAso[:, j * 272:(j + 1) * 272], h2[:])
                    else:
                        nc.scalar.copy(
                            hAso[:, j * 272:(j + 1) * 272], h2[:])
                    nc.tensor.matmul(sp[0:112, ASD_ + j * 8:ASD_ + j * 8 + 8],
                                     sblk_l[j], hAv[:, j, 256:264],
                                     start=True, stop=False)
                    nc.tensor.matmul(sp[0:112, ASD_ + j * 8:ASD_ + j * 8 + 8],
                                     dblk_l[j], hAv[:, j, 264:272],
                                     start=False, stop=True)
                    if li == 1:
                        nc.tensor.matmul(
                            sp[:, CNT_ + j * 2:CNT_ + j * 2 + 2], dtt_l[j][:],
                            c_eap[:, 2 * (p0 + j):2 * (p0 + j) + 2],
                            start=True, stop=True)

                if li == 1:
                    cntv = sp[:, CNT_:CNT_ + 16].rearrange(
                        "p (pr two) -> p pr two", two=2)
                    cntm = wk.tile([128, 8], F32, tag="cntm")
                    nc.vector.tensor_scalar(cntm[:], cntv[:, :, 1:2], 1.0,
                                            None, ALU.max)
                    rc = wk.tile([128, 8], F32, tag="rc")
                    nc.vector.reciprocal(rc[:], cntm[:])
                    ea_loop = wk.tile([128, 8], F32, tag="ea_loop")
                    nc.vector.tensor_tensor(ea_loop[:], cntv[:, :, 0:1]
                                            .rearrange("p a b -> p (a b)"),
                                            rc[:], ALU.mult)

                # ---- batched edge logits ----
                ae = wk.tile([EPP, 64], F32, tag="ae")
                nc.gpsimd.tensor_tensor(
                    ae[:].rearrange("p (a h) -> p a h", a=8),
                    c_webe[:, (li - 1) * 64:li * 64]
                    .rearrange("p (a h) -> p a h", a=8),
                    c_ea[:, p0:p0 + 8][:, :, None].broadcast_to([EPP, 8, H]),
                    ALU.mult)
                lg = wk.tile([EPP, 64], F32, tag="lg")
                nc.vector.scalar_tensor_tensor(
                    lg[:], sp[0:112, ASD_:ASD_ + 64], 1.0, ae[:],
                    ALU.mult, ALU.add)
                lg2 = wk.tile([EPP, 64], F32, tag="lg2")
                nc.vector.scalar_tensor_tensor(
                    lg2[:], lg[:], 0.2, lg[:], ALU.mult, ALU.max)
                p_e = wk.tile([EPP, 64], vdt, tag="p_e")
                nc.scalar.activation(p_e[:], lg2[:], ACTF.Exp)

                p_self = None
                if self_loops:
                    sae = wk.tile([128, 64], F32, tag="sae")
                    nc.gpsimd.tensor_tensor(
                        sae[:].rearrange("p (a h) -> p a h", a=8),
                        c_webn[:, (li - 1) * 64:li * 64]
                        .rearrange("p (a h) -> p a h", a=8),
                        ea_loop[:][:, :, None].broadcast_to([128, 8, H]),
                        ALU.mult)
                    s1 = wk.tile([128, 64], F32, tag="s1")
                    nc.gpsimd.tensor_tensor(
                        s1[:].rearrange("p (a h) -> p a h", a=8),
                        hAv[:, :, 256:264],
                        hAv[:, :, 264:272], ALU.add)
                    s2 = wk.tile([128, 64], F32, tag="s2")
                    nc.gpsimd.tensor_tensor(
                        s2[:], s1[:], sae[:], ALU.add)
                    s3 = wk.tile([128, 64], F32, tag="s3")
                    nc.vector.scalar_tensor_tensor(
                        s3[:], s2[:], 0.2, s2[:], ALU.mult, ALU.max)
                    p_self = wk.tile([128, 64], F32, tag="p_self")
                    nc.scalar.activation(p_self[:], s3[:], ACTF.Exp)

                # ---- denominators ----
                for j in range(8):
                    nc.tensor.matmul(sp[:, DEN_ + j * 8:DEN_ + j * 8 + 8],
                                     dtt_l[j][:], p_e[:, j * 8:j * 8 + 8],
                                     start=True, stop=True)
                dtot = wk.tile([128, 64], F32, tag="dtot")
                if self_loops:
                    nc.vector.tensor_tensor(dtot[:], sp[:, DEN_:DEN_ + 64],
                                            p_self[:], ALU.add)
                else:
                    nc.vector.tensor_scalar(dtot[:], sp[:, DEN_:DEN_ + 64],
                                            1e-16, None, ALU.add)
                rden = wk.tile([128, 64], F32, tag="rden")
                nc.vector.reciprocal(rden[:], dtot[:])
                rden_v = wk.tile([128, 64], vdt, tag="rden_v")
                nc.scalar.copy(rden_v[:], rden[:])
                for j in range(8):
                    nc.tensor.matmul(sp[0:112, RD_ + j * 8:RD_ + j * 8 + 8],
                                     dblk_l[j], rden_v[:, j * 8:j * 8 + 8],
                                     start=True, stop=True)
                pn = wk.tile([EPP, 64], vdt, tag="pn")
                nc.vector.tensor_tensor(pn[:], sp[0:112, RD_:RD_ + 64], p_e[:],
                                        ALU.mult)
                if self_loops:
                    psn = wk.tile([128, 64], vdt, tag="psn")
                    nc.gpsimd.tensor_tensor(psn[:], p_self[:], rden[:], ALU.mult)

                # ---- phase B per duet: gather h, messages, scatter, relu ----
                hT_new = []
                for d in range(4):
                    g2 = ps.tile([EPP, 512], F32, tag="g2", bufs=2)
                    for jj in range(2):
                        j = 2 * d + jj
                        nc.tensor.matmul(g2[:, jj * 256:jj * 256 + 256],
                                         sblk_l[j], hAv[:, j, 0:256],
                                         start=True, stop=True)
                    msg2 = wk.tile([EPP, 512], vdt, tag="msg2")
                    nc.vector.tensor_tensor(
                        msg2[:].rearrange("p (a h c) -> p a h c", a=2, h=H),
                        g2[:, :].rearrange("p (a h c) -> p a h c", a=2, h=H),
                        pn[:, d * 16:(d + 1) * 16]
                        .rearrange("p (a h) -> p a h", a=2)[:, :, :, None]
                        .broadcast_to([EPP, 2, H, HID]), ALU.mult)
                    if self_loops:
                        msgs2 = wk.tile([128, 512], vdt, tag="msgs2")
                        for jj in range(2):
                            j = 2 * d + jj
                            nc.gpsimd.tensor_tensor(
                                msgs2[:, jj * 256:(jj + 1) * 256]
                                .rearrange("p (h c) -> p h c", h=H),
                                hAv[:, j, 0:256]
                                .rearrange("p (h c) -> p h c", h=H),
                                psn[:, j * 8:(j + 1) * 8][:, :, None]
                                .broadcast_to([128, H, HID]), ALU.mult)
                    o2 = ps.tile([128, 512], F32, tag="out2", bufs=1)
                    for jj in range(2):
                        j = 2 * d + jj
                        for c in range(2):
                            cs = slice(jj * 256 + c * 128, jj * 256 + c * 128 + 128)
                            nc.tensor.matmul(
                                o2[:, cs], msg2[:, jj * 256 + c * 128:
                                                jj * 256 + (c + 1) * 128],
                                dtt_l[j][:], start=True, stop=not self_loops)
                            if self_loops:
                                nc.tensor.matmul(
                                    o2[:, cs], msgs2[:, jj * 256 + c * 128:
                                                     jj * 256 + (c + 1) * 128],
                                    c_id[:, :], start=False, stop=True)
                    hT2 = wk.tile([128, 512], vdt, tag=f"hT{li}_{d}")
                    nc.scalar.activation(hT2[:], o2[:], ACTF.Relu)
                    hT_new.append(hT2)
                    for jj in range(2):
                        j = 2 * d + jj
                        prevT[j] = [hT2[:, jj * 256:jj * 256 + 128],
                                    hT2[:, jj * 256 + 128:jj * 256 + 256]]
                hT_l3 = hT_new

            # ---- pooling + MLP (octet-batched) ----
            zmlp = ps.tile([128, 144], F32, tag="g2", bufs=2)
            gev_o = wk.tile([128, 32], F32, tag="gev_o")   # (c, pair, g)
            gvv = gev_o[:, :].rearrange("p (c pr g) -> p pr c g", c=2, g=2)
            for d in range(4):
                nc.vector.tensor_reduce(
                    gvv[:, 2 * d:2 * d + 2, :, :],
                    hT_l3[d][:, :].rearrange(
                        "p (a b g n) -> p a b g n", a=2, b=2, g=2),
                    mybir.AxisListType.X, ALU.add)
            gev_v = wk.tile([128, 32], vdt, tag="gev_v")
            nc.scalar.copy(gev_v[:], gev_o[:])
            for d in range(4):
                for c in range(2):
                    agent = hT_l3[d][:, :].rearrange(
                        "p (a b g n) -> p a b g n", a=2, b=2, g=2)[:, :, c, :, 0:8]
                    nc.tensor.matmul(zmlp[:, Z_ + d * 32:Z_ + (d + 1) * 32],
                                     c_fc1a[:, bass.ts(c, 128)], agent,
                                     start=(c == 0), stop=(c == 1))
            for c in range(2):
                nc.tensor.matmul(zmlp[:, ZG_:ZG_ + 16],
                                 c_fc1g[:, bass.ts(c, 128)],
                                 gev_v[:, c * 16:(c + 1) * 16],
                                 start=(c == 0), stop=(c == 1))
            zgb = wk.tile([128, 16], F32, tag="zgb")
            nc.vector.scalar_tensor_tensor(
                zgb[:], zmlp[:, ZG_:ZG_ + 16], 1.0,
                c_fc1b[:, 0:1].broadcast_to([128, 16]), ALU.mult, ALU.add)
            zt = wk.tile([128, 128], F32, tag="zt")
            nc.vector.scalar_tensor_tensor(
                zt[:].rearrange("p (a b) -> p a b", a=16),
                zmlp[:, Z_:Z_ + 128].rearrange("p (a b) -> p a b", a=16), 1.0,
                zgb[:][:, :, None].broadcast_to([128, 16, 8]),
                ALU.mult, ALU.add)
            zbat = wk.tile([128, 128], vdt, tag="zbat")
            nc.scalar.activation(zbat[:], zt[:], ACTF.Relu)
            nc.tensor.matmul(zmlp[0:OUT, 0:128], c_fc2w[:, :], zbat[:],
                             start=True, stop=True)
            nc.vector.tensor_scalar(out_acc[:, oct_i * 128:(oct_i + 1) * 128],
                                    zmlp[0:OUT, 0:128], c_fc2b[:, 0:1], None,
                                    ALU.add)

        nc.sync.dma_start(out_d[:, :], out_acc[:])

    nc.compile()
    return nc


# ---------------- host-side packing ----------------

def _np_vdt(vdt):
    import ml_dtypes
    return {mybir.dt.bfloat16: ml_dtypes.bfloat16,
            mybir.dt.float32: np.float32}[vdt]


def host_prep(inputs, npairs=GPC // 2, vdt=mybir.dt.bfloat16):
    nv = _np_vdt(vdt)
    x = np.asarray(inputs["x"], np.float32)
    ei = np.asarray(inputs["edge_index"])
    eattr = np.asarray(inputs["edge_attr"], np.float32)
    for l in (1, 2, 3):
        assert not np.any(np.asarray(inputs[f"b{l}"])), "GAT bias must be 0"

    def pack_w(l):
        W = np.asarray(inputs[f"W{l}"], np.float32)
        a_s = np.asarray(inputs[f"as{l}"], np.float32)
        a_d = np.asarray(inputs[f"ad{l}"], np.float32)
        Ps = np.einsum("fkc,kc->fk", W.reshape(W.shape[0], H, HID), a_s)
        Pd = np.einsum("fkc,kc->fk", W.reshape(W.shape[0], H, HID), a_d)
        return np.concatenate([W, Ps, Pd], axis=1).astype(nv)

    def w_e(l):
        We = np.asarray(inputs[f"We{l}"], np.float32).reshape(H, HID)
        a_e = np.asarray(inputs[f"ae{l}"], np.float32)
        return (We * a_e).sum(-1)

    waug = {l: pack_w(l) for l in (1, 2, 3)}
    for l in (2, 3):
        waug[l] = np.concatenate([waug[l][:128], waug[l][128:]], axis=1)
    wev = np.concatenate([np.tile(w_e(l), 8) for l in (1, 2, 3)])   # [192]
    webe = np.broadcast_to(wev, (EPP, 192)).astype(np.float32).copy()
    webn = np.broadcast_to(wev, (128, 192)).astype(np.float32).copy()
    fc1_w = np.asarray(inputs["fc1_w"], np.float32)
    fc1a = np.concatenate([fc1_w[:128], fc1_w[128:HC]], axis=1).astype(nv)
    fc1g = np.concatenate([fc1_w[HC:HC + 128] / P,
                           fc1_w[HC + 128:] / P], axis=1).astype(nv)
    fc1b = np.asarray(inputs["fc1_b"], np.float32).reshape(128, 1)
    fc2w = np.asarray(inputs["fc2_w"], np.float32).astype(nv)
    fc2b = np.asarray(inputs["fc2_b"], np.float32).reshape(OUT, 1)
    identm = np.eye(128, dtype=np.float32).astype(nv)
    iota = np.broadcast_to(np.arange(128, dtype=np.float32),
                           (EPP, 128)).astype(nv).copy()
    iotac = np.arange(128, dtype=np.float32).reshape(128, 1)

    maps = []
    npc = GPC * P
    epc = GPC * OBS
    for m in range(NCORES):
        nsl = slice(m * npc, (m + 1) * npc)
        esl = slice(m * epc, (m + 1) * epc)
        xt = np.ascontiguousarray(x[nsl].T).astype(nv)
        src = np.asarray(ei[0][esl], np.int64) - m * npc
        dst = np.asarray(ei[1][esl], np.int64) - m * npc
        pairs = np.arange(GPC // 2).repeat(EPP)
        src_l = (src.reshape(-1) - pairs * 128).astype(np.float32)
        dst_l = (dst.reshape(-1) - pairs * 128).astype(np.float32)
        esrcb = np.ascontiguousarray(src_l.reshape(-1, EPP)).astype(nv)
        edstb = np.ascontiguousarray(dst_l.reshape(-1, EPP)).astype(nv)
        edst = np.ascontiguousarray(dst_l.reshape(-1, EPP).T)
        eat = np.ascontiguousarray(eattr[esl].reshape(-1, EPP).T).astype(np.float32)
        eap_arr = np.empty((EPP, 2 * npairs), np.float32)
        eap_arr[:, 0::2] = eat[:, :npairs]
        eap_arr[:, 1::2] = 1.0
        maps.append({
            "xt": xt[:, :npairs * 128],
            "esrcb": esrcb[:npairs], "edstb": edstb[:npairs],
            "edst": edst[:, :npairs],
            "eattr": eat[:, :npairs], "eap": eap_arr.astype(nv),
            "waug1": waug[1], "waug2": waug[2], "waug3": waug[3],
            "webe": webe, "webn": webn,
            "fc1a": fc1a, "fc1g": fc1g, "fc1b": fc1b,
            "fc2w": fc2w, "fc2b": fc2b,
            "ident": identm, "iota": iota, "iotac": iotac,
        })
    return maps


def unpack_out(res_list, npairs=GPC // 2):
    outs = []
    for m in range(NCORES):
        o = res_list[m]["out"]
        o = o.reshape(OUT, npairs, 2, A).transpose(1, 2, 3, 0)
        outs.append(o.reshape(npairs * 2, A, OUT))
    return np.concatenate(outs, axis=0).astype(np.float32)


# ---------------- entry point ----------------

LAST_EXEC_NS = None
_NC_CACHE = {}


def kernel(**inputs) -> np.ndarray:
    """Full-input GAT forward on 8 NeuronCores; returns [4096, 8, 2] f32."""
    global LAST_EXEC_NS
    import os
    vdt = mybir.dt.bfloat16
    npairs = GPC // 2
    key = (npairs, vdt)
    if key not in _NC_CACHE:
        _NC_CACHE[key] = build(npairs, vdt=vdt, num_devices=NCORES)
    nc = _NC_CACHE[key]
    maps = host_prep(inputs, npairs=npairs, vdt=vdt)
    trace = os.environ.get("BASS_GAT_TRACE") == "1"
    res = None
    for attempt in range(3):
        try:
            res = run_bass_kernel_spmd(
                nc, maps, core_ids=list(range(NCORES)),
                trace=trace and attempt == 0,
                trace_cores=[0] if trace and attempt == 0 else None)
            break
        except Exception:
            if attempt == 2:
                raise
            import time
            time.sleep(10)
    LAST_EXEC_NS = res.exec_time_ns
    return unpack_out([r for r in res.results], npairs=npairs)


# revision 7
# speedup vs baseline: 1.5618x; 1.0004x over previous
"""GAT model Bass/Tile kernel for TRN2 (self-contained, octet-batched).

Per core: 512 graphs as 256 pairs (128 nodes / 112 edges). Pairs are
processed in octets (8 pairs): per-edge/per-node attention scalars are
batched into [*, 64] ops across the octet; fat value ops run at duet
(2-pair) granularity; engines are balanced DVE/ACT/GPSIMD/PE.
"""
import numpy as np
from contextlib import ExitStack

import concourse.bass as bass
import concourse.tile as tile
from concourse import bacc, mybir
from concourse.bass_utils import run_bass_kernel_spmd

F32 = mybir.dt.float32
I32 = mybir.dt.int32

B, A, OBS = 4096, 8, 56
P = 64
H, HID, HC = 8, 32, 256
IN, OUT = 16, 2
NCORES = 8
GPC = B // NCORES
EPP = 2 * OBS
ALU = mybir.AluOpType
ACTF = mybir.ActivationFunctionType

# small_ps column regions (f32); Z/ZG/oc reuse the same tile post-L3
ASD_, DEN_, RD_, CNT_, Z_, ZG_ = 0, 64, 128, 192, 0, 128


def build(npairs: int, vdt=mybir.dt.bfloat16, num_devices: int = NCORES):
    assert npairs % 8 == 0
    nc = bacc.Bacc("TRN2", target_bir_lowering=False, debug=False,
                   num_devices=num_devices)
    NP = npairs

    def din(name, shape, dt):
        return nc.dram_tensor(name, shape, dt, kind="ExternalInput").ap()

    xt = din("xt", [IN, NP * 128], vdt)
    esrcb = din("esrcb", [NP, EPP], vdt)
    edstb = din("edstb", [NP, EPP], vdt)
    edst = din("edst", [EPP, NP], F32)
    eattr = din("eattr", [EPP, NP], F32)
    eap = din("eap", [EPP, 2 * NP], vdt)
    waug1 = din("waug1", [IN, 272], vdt)
    waug2 = din("waug2", [128, 544], vdt)
    waug3 = din("waug3", [128, 544], vdt)
    webe = din("webe", [EPP, 3 * 64], F32)    # w_e tiled 8x per layer
    webn = din("webn", [128, 3 * 64], F32)
    fc1a = din("fc1a", [128, HC], vdt)
    fc1g = din("fc1g", [128, HC], vdt)
    fc1b = din("fc1b", [128, 1], F32)
    fc2w = din("fc2w", [128, OUT], vdt)
    fc2b = din("fc2b", [OUT, 1], F32)
    ident = din("ident", [128, 128], vdt)
    iota = din("iota", [EPP, 128], vdt)
    iotac = din("iotac", [128, 1], F32)

    out_d = nc.dram_tensor("out", [OUT, NP * 16], F32, kind="ExternalOutput").ap()

    with tile.TileContext(nc) as tc, ExitStack() as ctx:
        cpool = ctx.enter_context(tc.tile_pool(name="const", bufs=1))
        wk = ctx.enter_context(tc.tile_pool(name="work", bufs=4))
        eb = ctx.enter_context(tc.tile_pool(name="edges", bufs=24))
        ps = ctx.enter_context(tc.tile_pool(name="psum", bufs=1, space="PSUM"))

        def cload(ap, tag):
            t = cpool.tile(list(ap.shape), ap.dtype, tag=tag)
            nc.sync.dma_start(t[:], ap[:, :])
            return t

        c_w1, c_w2, c_w3 = cload(waug1, "w1"), cload(waug2, "w2"), cload(waug3, "w3")
        c_webe, c_webn = cload(webe, "webe"), cload(webn, "webn")
        c_fc1a, c_fc1g = cload(fc1a, "fc1a"), cload(fc1g, "fc1g")
        c_fc1b, c_fc2w, c_fc2b = cload(fc1b, "fc1b"), cload(fc2w, "fc2w"), cload(fc2b, "fc2b")
        c_id, c_iota = cload(ident, "ident"), cload(iota, "iota")
        c_iotac = cload(iotac, "iotac")
        c_edst = cload(edst, "edst")
        c_ea, c_eap = cload(eattr, "eattr"), cload(eap, "eap")

        out_acc = cpool.tile([OUT, NP * 16], F32, tag="out_acc")

        wchunks = {1: [c_w1[:, :]],
                   2: [c_w2[:, 0:272], c_w2[:, 272:544]],
                   3: [c_w3[:, 0:272], c_w3[:, 272:544]]}

        for oct_i in range(NP // 8):
            p0 = oct_i * 8

            # ---- phase A: edge structure + x loads ----
            srcb = eb.tile([128, 8 * EPP], vdt, tag="srcb", bufs=2)
            nc.sync.dma_start(srcb[:], esrcb[p0:p0 + 8, :]
                              .rearrange("a b -> (a b)")[None, :]
                              .broadcast_to([128, 8 * EPP]))
            dstb = eb.tile([128, 8 * EPP], vdt, tag="dstb", bufs=2)
            nc.sync.dma_start(dstb[:], edstb[p0:p0 + 8, :]
                              .rearrange("a b -> (a b)")[None, :]
                              .broadcast_to([128, 8 * EPP]))
            sblk_o = eb.tile([128, 8 * EPP], vdt, tag="sblk_o", bufs=2)
            nc.vector.tensor_scalar(sblk_o[:], srcb[:], c_iotac[:, 0:1],
                                    None, ALU.is_equal)
            dblk_o = eb.tile([128, 8 * EPP], vdt, tag="dblk_o", bufs=2)
            nc.vector.tensor_scalar(dblk_o[:], dstb[:], c_iotac[:, 0:1],
                                    None, ALU.is_equal)
            sblk_l = [sblk_o[:, j * EPP:(j + 1) * EPP] for j in range(8)]
            dblk_l = [dblk_o[:, j * EPP:(j + 1) * EPP] for j in range(8)]
            dtt_l, x0_l = [], []
            for j in range(8):
                pp = p0 + j
                dtt = eb.tile([EPP, 128], vdt, tag="dtt")
                nc.vector.tensor_scalar(dtt[:], c_iota[:], c_edst[:, pp:pp + 1],
                                        None, ALU.is_equal)
                x0 = eb.tile([IN, 128], vdt, tag="x0")
                nc.sync.dma_start(x0[:], xt[:, pp * 128:(pp + 1) * 128])
                dtt_l.append(dtt); x0_l.append(x0)

            prevT = [[x0_l[j][:, :]] for j in range(8)]
            ea_loop = None
            hT_l3 = None

            for li in (1, 2, 3):
                wch = wchunks[li]
                self_loops = li > 1
                sp = ps.tile([128, 208], F32, tag="small", bufs=2)
                hAso = wk.tile([128, 8 * 272], vdt, tag="hAso", bufs=3)
                hAv = hAso[:, :].rearrange("p (pr x) -> p pr x", pr=8)

                # ---- MM1 per pair + hAs copy + small gathers ----
                for j in range(8):
                    h2 = ps.tile([128, 272], F32, tag="h2", bufs=3)
                    for kc, (sta, wc) in enumerate(zip(prevT[j], wch)):
                        nc.tensor.matmul(h2[:], sta, wc, start=(kc == 0),
                                         stop=(kc == len(wch) - 1))
                    if j % 4 == 0:
                        nc.vector.tensor_copy(
                            hAso[:, j * 272:(j + 1) * 272], h2[:])
                    else:
                        nc.scalar.copy(
                            hAso[:, j * 272:(j + 1) * 272], h2[:])
                    nc.tensor.matmul(sp[0:112, ASD_ + j * 8:ASD_ + j * 8 + 8],
                                     sblk_l[j], hAv[:, j, 256:264],
                                     start=True, stop=False)
                    nc.tensor.matmul(sp[0:112, ASD_ + j * 8:ASD_ + j * 8 + 8],
                                     dblk_l[j], hAv[:, j, 264:272],
                                     start=False, stop=True)
                    if li == 1:
                        nc.tensor.matmul(
                            sp[:, CNT_ + j * 2:CNT_ + j * 2 + 2], dtt_l[j][:],
                            c_eap[:, 2 * (p0 + j):2 * (p0 + j) + 2],
                            start=True, stop=True)

                if li == 1:
                    cntv = sp[:, CNT_:CNT_ + 16].rearrange(
                        "p (pr two) -> p pr two", two=2)
                    cntm = wk.tile([128, 8], F32, tag="cntm")
                    nc.vector.tensor_scalar(cntm[:], cntv[:, :, 1:2], 1.0,
                                            None, ALU.max)
                    rc = wk.tile([128, 8], F32, tag="rc")
                    nc.vector.reciprocal(rc[:], cntm[:])
                    ea_loop = wk.tile([128, 8], F32, tag="ea_loop")
                    nc.vector.tensor_tensor(ea_loop[:], cntv[:, :, 0:1]
                                            .rearrange("p a b -> p (a b)"),
                                            rc[:], ALU.mult)

                # ---- batched edge logits ----
                ae = wk.tile([EPP, 64], F32, tag="ae")
                nc.gpsimd.tensor_tensor(
                    ae[:].rearrange("p (a h) -> p a h", a=8),
                    c_webe[:, (li - 1) * 64:li * 64]
                    .rearrange("p (a h) -> p a h", a=8),
                    c_ea[:, p0:p0 + 8][:, :, None].broadcast_to([EPP, 8, H]),
                    ALU.mult)
                lg = wk.tile([EPP, 64], F32, tag="lg")
                nc.vector.scalar_tensor_tensor(
                    lg[:], sp[0:112, ASD_:ASD_ + 64], 1.0, ae[:],
                    ALU.mult, ALU.add)
                lg2 = wk.tile([EPP, 64], F32, tag="lg2")
                nc.vector.scalar_tensor_tensor(
                    lg2[:], lg[:], 0.2, lg[:], ALU.mult, ALU.max)
                p_e = wk.tile([EPP, 64], vdt, tag="p_e")
                nc.scalar.activation(p_e[:], lg2[:], ACTF.Exp)

                p_self = None
                if self_loops:
                    sae = wk.tile([128, 64], F32, tag="sae")
                    nc.gpsimd.tensor_tensor(
                        sae[:].rearrange("p (a h) -> p a h", a=8),
                        c_webn[:, (li - 1) * 64:li * 64]
                        .rearrange("p (a h) -> p a h", a=8),
                        ea_loop[:][:, :, None].broadcast_to([128, 8, H]),
                        ALU.mult)
                    s1 = wk.tile([128, 64], F32, tag="s1")
                    nc.gpsimd.tensor_tensor(
                        s1[:].rearrange("p (a h) -> p a h", a=8),
                        hAv[:, :, 256:264],
                        hAv[:, :, 264:272], ALU.add)
                    s2 = wk.tile([128, 64], F32, tag="s2")
                    nc.gpsimd.tensor_tensor(
                        s2[:], s1[:], sae[:], ALU.add)
                    s3 = wk.tile([128, 64], F32, tag="s3")
                    nc.vector.scalar_tensor_tensor(
                        s3[:], s2[:], 0.2, s2[:], ALU.mult, ALU.max)
                    p_self = wk.tile([128, 64], F32, tag="p_self")
                    nc.scalar.activation(p_self[:], s3[:], ACTF.Exp)

                # ---- denominators ----
                for j in range(8):
                    nc.tensor.matmul(sp[:, DEN_ + j * 8:DEN_ + j * 8 + 8],
                                     dtt_l[j][:], p_e[:, j * 8:j * 8 + 8],
                                     start=True, stop=True)
                dtot = wk.tile([128, 64], F32, tag="dtot")
                if self_loops:
                    nc.vector.tensor_tensor(dtot[:], sp[:, DEN_:DEN_ + 64],
                                            p_self[:], ALU.add)
                else:
                    nc.vector.tensor_scalar(dtot[:], sp[:, DEN_:DEN_ + 64],
                                            1e-16, None, ALU.add)
                rden = wk.tile([128, 64], F32, tag="rden")
                nc.vector.reciprocal(rden[:], dtot[:])
                rden_v = wk.tile([128, 64], vdt, tag="rden_v")
                nc.scalar.copy(rden_v[:], rden[:])
                for j in range(8):
                    nc.tensor.matmul(sp[0:112, RD_ + j * 8:RD_ + j * 8 + 8],
                                     dblk_l[j], rden_v[:, j * 8:j * 8 + 8],
                                     start=True, stop=True)
                pn = wk.tile([EPP, 64], vdt, tag="pn")
                nc.vector.tensor_tensor(pn[:], sp[0:112, RD_:RD_ + 64], p_e[:],
                                        ALU.mult)
                if self_loops:
                    psn = wk.tile([128, 64], vdt, tag="psn")
                    nc.gpsimd.tensor_tensor(psn[:], p_self[:], rden[:], ALU.mult)

                # ---- phase B per duet: gather h, messages, scatter, relu ----
                hT_new = []
                for d in range(4):
                    g2 = ps.tile([EPP, 512], F32, tag="g2", bufs=2)
                    for jj in range(2):
                        j = 2 * d + jj
                        nc.tensor.matmul(g2[:, jj * 256:jj * 256 + 256],
                                         sblk_l[j], hAv[:, j, 0:256],
                                         start=True, stop=True)
                    msg2 = wk.tile([EPP, 512], vdt, tag="msg2")
                    nc.vector.tensor_tensor(
                        msg2[:].rearrange("p (a h c) -> p a h c", a=2, h=H),
                        g2[:, :].rearrange("p (a h c) -> p a h c", a=2, h=H),
                        pn[:, d * 16:(d + 1) * 16]
                        .rearrange("p (a h) -> p a h", a=2)[:, :, :, None]
                        .broadcast_to([EPP, 2, H, HID]), ALU.mult)
                    if self_loops:
                        msgs2 = wk.tile([128, 512], vdt, tag="msgs2")
                        for jj in range(2):
                            j = 2 * d + jj
                            nc.gpsimd.tensor_tensor(
                                msgs2[:, jj * 256:(jj + 1) * 256]
                                .rearrange("p (h c) -> p h c", h=H),
                                hAv[:, j, 0:256]
                                .rearrange("p (h c) -> p h c", h=H),
                                psn[:, j * 8:(j + 1) * 8][:, :, None]
                                .broadcast_to([128, H, HID]), ALU.mult)
                    o2 = ps.tile([128, 512], F32, tag="out2", bufs=1)
                    for jj in range(2):
                        j = 2 * d + jj
                        for c in range(2):
                            cs = slice(jj * 256 + c * 128, jj * 256 + c * 128 + 128)
                            nc.tensor.matmul(
                                o2[:, cs], msg2[:, jj * 256 + c * 128:
                                                jj * 256 + (c + 1) * 128],
                                dtt_l[j][:], start=True, stop=not self_loops)
                            if self_loops:
                                nc.tensor.matmul(
                                    o2[:, cs], msgs2[:, jj * 256 + c * 128:
                                                     jj * 256 + (c + 1) * 128],
                                    c_id[:, :], start=False, stop=True)
                    hT2 = wk.tile([128, 512], vdt, tag=f"hT{li}_{d}")
                    nc.scalar.activation(hT2[:], o2[:], ACTF.Relu)
                    hT_new.append(hT2)
                    for jj in range(2):
                        j = 2 * d + jj
                        prevT[j] = [hT2[:, jj * 256:jj * 256 + 128],
                                    hT2[:, jj * 256 + 128:jj * 256 + 256]]
                hT_l3 = hT_new

            # ---- pooling + MLP (octet-batched) ----
            zmlp = ps.tile([128, 144], F32, tag="g2", bufs=2)
            gev_o = wk.tile([128, 32], F32, tag="gev_o")   # (c, pair, g)
            gvv = gev_o[:, :].rearrange("p (c pr g) -> p pr c g", c=2, g=2)
            for d in range(4):
                nc.vector.tensor_reduce(
                    gvv[:, 2 * d:2 * d + 2, :, :],
                    hT_l3[d][:, :].rearrange(
                        "p (a b g n) -> p a b g n", a=2, b=2, g=2),
                    mybir.AxisListType.X, ALU.add)
            gev_v = wk.tile([128, 32], vdt, tag="gev_v")
            nc.scalar.copy(gev_v[:], gev_o[:])
            for d in range(4):
                for c in range(2):
                    agent = hT_l3[d][:, :].rearrange(
                        "p (a b g n) -> p a b g n", a=2, b=2, g=2)[:, :, c, :, 0:8]
                    nc.tensor.matmul(zmlp[:, Z_ + d * 32:Z_ + (d + 1) * 32],
                                     c_fc1a[:, bass.ts(c, 128)], agent,
                                     start=(c == 0), stop=(c == 1))
            for c in range(2):
                nc.tensor.matmul(zmlp[:, ZG_:ZG_ + 16],
                                 c_fc1g[:, bass.ts(c, 128)],
                                 gev_v[:, c * 16:(c + 1) * 16],
                                 start=(c == 0), stop=(c == 1))
            zgb = wk.tile([128, 16], F32, tag="zgb")
            nc.vector.scalar_tensor_tensor(
                zgb[:], zmlp[:, ZG_:ZG_ + 16], 1.0,
                c_fc1b[:, 0:1].broadcast_to([128, 16]), ALU.mult, ALU.add)
            zt = wk.tile([128, 128], F32, tag="zt")
            nc.vector.scalar_tensor_tensor(
                zt[:].rearrange("p (a b) -> p a b", a=16),
                zmlp[:, Z_:Z_ + 128].rearrange("p (a b) -> p a b", a=16), 1.0,
                zgb[:][:, :, None].broadcast_to([128, 16, 8]),
                ALU.mult, ALU.add)
            zbat = wk.tile([128, 128], vdt, tag="zbat")
            nc.scalar.activation(zbat[:], zt[:], ACTF.Relu)
            nc.tensor.matmul(zmlp[0:OUT, 0:128], c_fc2w[:, :], zbat[:],
                             start=True, stop=True)
            nc.vector.tensor_scalar(out_acc[:, oct_i * 128:(oct_i + 1) * 128],
                                    zmlp[0:OUT, 0:128], c_fc2b[:, 0:1], None,
                                    ALU.add)

        nc.sync.dma_start(out_d[:, :], out_acc[:])

    nc.compile()
    return nc


# ---------------- host-side packing ----------------

def _np_vdt(vdt):
    import ml_dtypes
    return {mybir.dt.bfloat16: ml_dtypes.bfloat16,
            mybir.dt.float32: np.float32}[vdt]


def host_prep(inputs, npairs=GPC // 2, vdt=mybir.dt.bfloat16):
    nv = _np_vdt(vdt)
    x = np.asarray(inputs["x"], np.float32)
    ei = np.asarray(inputs["edge_index"])
    eattr = np.asarray(inputs["edge_attr"], np.float32)
    for l in (1, 2, 3):
        assert not np.any(np.asarray(inputs[f"b{l}"])), "GAT bias must be 0"

    def pack_w(l):
        W = np.asarray(inputs[f"W{l}"], np.float32)
        a_s = np.asarray(inputs[f"as{l}"], np.float32)
        a_d = np.asarray(inputs[f"ad{l}"], np.float32)
        Ps = np.einsum("fkc,kc->fk", W.reshape(W.shape[0], H, HID), a_s)
        Pd = np.einsum("fkc,kc->fk", W.reshape(W.shape[0], H, HID), a_d)
        return np.concatenate([W, Ps, Pd], axis=1).astype(nv)

    def w_e(l):
        We = np.asarray(inputs[f"We{l}"], np.float32).reshape(H, HID)
        a_e = np.asarray(inputs[f"ae{l}"], np.float32)
        return (We * a_e).sum(-1)

    waug = {l: pack_w(l) for l in (1, 2, 3)}
    for l in (2, 3):
        waug[l] = np.concatenate([waug[l][:128], waug[l][128:]], axis=1)
    wev = np.concatenate([np.tile(w_e(l), 8) for l in (1, 2, 3)])   # [192]
    webe = np.broadcast_to(wev, (EPP, 192)).astype(np.float32).copy()
    webn = np.broadcast_to(wev, (128, 192)).astype(np.float32).copy()
    fc1_w = np.asarray(inputs["fc1_w"], np.float32)
    fc1a = np.concatenate([fc1_w[:128], fc1_w[128:HC]], axis=1).astype(nv)
    fc1g = np.concatenate([fc1_w[HC:HC + 128] / P,
                           fc1_w[HC + 128:] / P], axis=1).astype(nv)
    fc1b = np.asarray(inputs["fc1_b"], np.float32).reshape(128, 1)
    fc2w = np.asarray(inputs["fc2_w"], np.float32).astype(nv)
    fc2b = np.asarray(inputs["fc2_b"], np.float32).reshape(OUT, 1)
    identm = np.eye(128, dtype=np.float32).astype(nv)
    iota = np.broadcast_to(np.arange(128, dtype=np.float32),
                           (EPP, 128)).astype(nv).copy()
    iotac = np.arange(128, dtype=np.float32).reshape(128, 1)

    maps = []
    npc = GPC * P
    epc = GPC * OBS
    for m in range(NCORES):
        nsl = slice(m * npc, (m + 1) * npc)
        esl = slice(m * epc, (m + 1) * epc)
        xt = np.ascontiguousarray(x[nsl].T).astype(nv)
        src = np.asarray(ei[0][esl], np.int64) - m * npc
        dst = np.asarray(ei[1][esl], np.int64) - m * npc
        pairs = np.arange(GPC // 2).repeat(EPP)
        src_l = (src.reshape(-1) - pairs * 128).astype(np.float32)
        dst_l = (dst.reshape(-1) - pairs * 128).astype(np.float32)
        esrcb = np.ascontiguousarray(src_l.reshape(-1, EPP)).astype(nv)
        edstb = np.ascontiguousarray(dst_l.reshape(-1, EPP)).astype(nv)
        edst = np.ascontiguousarray(dst_l.reshape(-1, EPP).T)
        eat = np.ascontiguousarray(eattr[esl].reshape(-1, EPP).T).astype(np.float32)
        eap_arr = np.empty((EPP, 2 * npairs), np.float32)
        eap_arr[:, 0::2] = eat[:, :npairs]
        eap_arr[:, 1::2] = 1.0
        maps.append({
            "xt": xt[:, :npairs * 128],
            "esrcb": esrcb[:npairs], "edstb": edstb[:npairs],
            "edst": edst[:, :npairs],
            "eattr": eat[:, :npairs], "eap": eap_arr.astype(nv),
            "waug1": waug[1], "waug2": waug[2], "waug3": waug[3],
            "webe": webe, "webn": webn,
            "fc1a": fc1a, "fc1g": fc1g, "fc1b": fc1b,
            "fc2w": fc2w, "fc2b": fc2b,
            "ident": identm, "iota": iota, "iotac": iotac,
        })
    return maps


def unpack_out(res_list, npairs=GPC // 2):
    outs = []
    for m in range(NCORES):
        o = res_list[m]["out"]
        o = o.reshape(OUT, npairs, 2, A).transpose(1, 2, 3, 0)
        outs.append(o.reshape(npairs * 2, A, OUT))
    return np.concatenate(outs, axis=0).astype(np.float32)


# ---------------- entry point ----------------

LAST_EXEC_NS = None
_NC_CACHE = {}


def kernel(**inputs) -> np.ndarray:
    """Full-input GAT forward on 8 NeuronCores; returns [4096, 8, 2] f32."""
    global LAST_EXEC_NS
    import os
    vdt = mybir.dt.bfloat16
    npairs = GPC // 2
    key = (npairs, vdt)
    if key not in _NC_CACHE:
        _NC_CACHE[key] = build(npairs, vdt=vdt, num_devices=NCORES)
    nc = _NC_CACHE[key]
    maps = host_prep(inputs, npairs=npairs, vdt=vdt)
    trace = os.environ.get("BASS_GAT_TRACE") == "1"
    res = None
    for attempt in range(3):
        try:
            res = run_bass_kernel_spmd(
                nc, maps, core_ids=list(range(NCORES)),
                trace=trace and attempt == 0,
                trace_cores=[0] if trace and attempt == 0 else None)
            break
        except Exception:
            if attempt == 2:
                raise
            import time
            time.sleep(10)
    LAST_EXEC_NS = res.exec_time_ns
    return unpack_out([r for r in res.results], npairs=npairs)
